# revision 5
# baseline (speedup 1.0000x reference)
"""GraphConv x2 + BN + ReLU + mean-pool + classifier on 8 TRN2 cores.

Strategy (dst-sharded nodes, segment-sum as one-hot matmul):
  - Nodes are split into 8 contiguous blocks of 12500 (padded to 12544 =
    98 chunks x 128).  Each core owns the edges whose dst falls in its block
    (edge-cut partitioning by dst).
  - Edges per core are grouped by 128-node dst-chunk, sorted by src inside
    the chunk, padded per-chunk to T=18 subchunks of 128 edges.
  - Aggregation m^T[feat, seg] += G^T S per 128-edge subchunk:
      G   [128 edges, 64] gathered rows of the (replicated) feature table
      S   [128 edges, 128 segs] one-hot built on DVE from iota==seg, scaled
          by w_e = rsqrt(deg_out[src]) * rsqrt(deg_in[dst])  (norm='both')
    so PSUM accumulates the normalized message sum transposed.
  - Per chunk: h^T = W^T m^T via a second matmul (the conv bias is
    dropped: BatchNorm right after is shift-invariant); BN partial sums;
    h^T written to HBM (pre-BN).
  - BatchNorm needs global stats -> separate transform launch per layer:
    reduces the 8 cores' partials, applies relu(a*h + c), transposes to
    row-major for the next layer's gather (or mean-pool + classifier at
    the end).
  - Host work between launches is routing only (concat / slicing);
    final output = sum of per-core partial logits / N + bc.

Launches: L1 agg(x, W1, b1) -> L2 transform1 -> L3 agg(h1, W2, b2) -> L4
transform2+readout.
"""
import sys

import numpy as np

sys.path.insert(0, "/opt/trn_rl_repo")

import concourse.bacc as bacc
import concourse.mybir as mybir
import concourse.tile as tile
from concourse.bass import IndirectOffsetOnAxis
from concourse.masks import make_identity

dt = mybir.dt

# ---- problem constants (fixed by the harness) ----
N = 100_000
E = 1_600_000
F = 64
NCORES = 8
P = 128
NPC = 12_500          # nodes per core
CH = 98               # 128-node chunks per core (98*128 = 12544)
NPAD = CH * P         # padded nodes per core
T = 18                # subchunks (of 128 edges) per chunk
EPS = 1e-5
SEG_PAD = 10_000.0    # seg id for pad edges (never matches iota 0..127)

_trace = {"on": False}


def _run(nc, in_maps, trace=None):
    from concourse.bass_utils import run_bass_kernel_spmd

    use_trace = _trace["on"] if trace is None else trace
    if use_trace:
        try:
            import ntff_hook

            ntff_hook.install()
        except Exception:
            use_trace = False
    res = run_bass_kernel_spmd(
        nc,
        in_maps,
        list(range(NCORES)),
        trace=use_trace,
        trace_cores=[0] if use_trace else None,
    )
    return res


# --------------------------------------------------------------------------
# Launch builders
# --------------------------------------------------------------------------

def build_agg(nc_cache={}):
    """Aggregation launch: gather + segment-matmul + W matmul + stat partials.

    Inputs per core:
      xin  [N_ROWS, 64] f32   feature table (replicated, padded rows)
      idx  [128, CH*T] i32    src id of edge (subchunk t, lane p)
      seg  [128, CH*T] f32    dst-local seg id (0..127) or SEG_PAD
      w    [128, CH*T] f32    edge weight (0 for pad)
      Wt   [64, 64]  f32      layer weight
      bt   [64, 1]   f32      layer bias
    Outputs:
      hpreT [64, NPAD] f32    pre-BN h, transposed (channels on partitions)
      stats [64, 2]   f32     [sum, sumsq] over this core's real nodes
    """
    if "agg" in nc_cache:
        return nc_cache["agg"]
    NROWS = N + 352  # 100352, multiple of 128
    nc = bacc.Bacc("TRN2", target_bir_lowering=False, debug=False)
    xin = nc.dram_tensor("xin", [NROWS, F], dt.float32, kind="ExternalInput")
    idx = nc.dram_tensor("idx", [P, CH * T], dt.int32, kind="ExternalInput")
    seg = nc.dram_tensor("seg", [P, CH * T], dt.float32, kind="ExternalInput")
    w = nc.dram_tensor("w", [P, CH * T], dt.float32, kind="ExternalInput")
    Wt = nc.dram_tensor("Wt", [F, F], dt.float32, kind="ExternalInput")
    hpreT = nc.dram_tensor("hpreT", [F, NPAD], dt.float32, kind="ExternalOutput")
    stats = nc.dram_tensor("stats", [F, 2], dt.float32, kind="ExternalOutput")

    with tile.TileContext(nc) as tc:
        with (
            tc.tile_pool(name="cp", bufs=1) as cp,
            tc.tile_pool(name="gp", bufs=3) as gp,
            tc.tile_pool(name="sp", bufs=3) as sp,
            tc.tile_pool(name="ep", bufs=2) as ep,
            tc.tile_pool(name="pp", bufs=2, space="PSUM") as pp,
        ):
            idx_t = cp.tile([P, CH * T], dt.int32)
            nc.sync.dma_start(out=idx_t[:], in_=idx[:])
            seg_t = cp.tile([P, CH * T], dt.float32)
            nc.sync.dma_start(out=seg_t[:], in_=seg[:])
            w_t = cp.tile([P, CH * T], dt.float32)
            nc.sync.dma_start(out=w_t[:], in_=w[:])
            W_t = cp.tile([F, F], dt.float32)
            nc.sync.dma_start(out=W_t[:], in_=Wt[:])

            iota_i = cp.tile([P, P], dt.int32)
            nc.gpsimd.iota(iota_i[:], pattern=[[1, P]], base=0, channel_multiplier=0)
            iota_f = cp.tile([P, P], dt.float32)
            nc.vector.tensor_copy(out=iota_f[:], in_=iota_i[:])

            sum_sb = cp.tile([F, CH], dt.float32)
            sq_sb = cp.tile([F, CH], dt.float32)

            for g in range(CH):
                G = gp.tile([P, T, F], dt.float32, tag="G")
                for t in range(T):
                    nc.gpsimd.indirect_dma_start(
                        out=G[:, t, :],
                        out_offset=None,
                        in_=xin[:],
                        in_offset=IndirectOffsetOnAxis(
                            ap=idx_t[:, g * T + t : g * T + t + 1], axis=0
                        ),
                    )
                mT_ps = pp.tile([F, P], dt.float32, tag="mT")
                for t in range(T):
                    S = sp.tile([P, P], dt.float32, tag="S")
                    nc.vector.tensor_scalar(
                        out=S[:],
                        in0=iota_f[:],
                        scalar1=seg_t[:, g * T + t : g * T + t + 1],
                        scalar2=w_t[:, g * T + t : g * T + t + 1],
                        op0=mybir.AluOpType.is_equal,
                        op1=mybir.AluOpType.mult,
                    )
                    nc.tensor.matmul(
                        out=mT_ps[:],
                        lhsT=G[:, t, :],
                        rhs=S[:],
                        start=(t == 0),
                        stop=(t == T - 1),
                    )
                mT_sb = ep.tile([F, P], dt.float32, tag="mTsb")
                nc.vector.tensor_copy(out=mT_sb[:], in_=mT_ps[:])
                hT_ps = pp.tile([F, P], dt.float32, tag="hT")
                nc.tensor.matmul(
                    out=hT_ps[:], lhsT=W_t[:], rhs=mT_sb[:], start=True, stop=True
                )
                # h = W^T m  (conv bias is BN-shift-invariant: dropped).
                # Pad node columns are exactly zero, so stats need no mask.
                hT_sb = ep.tile([F, P], dt.float32, tag="hTsb")
                nc.vector.tensor_copy(out=hT_sb[:], in_=hT_ps[:])
                nc.vector.reduce_sum(
                    out=sum_sb[:, g : g + 1], in_=hT_sb[:],
                    axis=mybir.AxisListType.X,
                )
                sq_scr = ep.tile([F, P], dt.float32, tag="sq")
                nc.scalar.activation(
                    out=sq_scr[:],
                    in_=hT_sb[:],
                    func=mybir.ActivationFunctionType.Square,
                    accum_out=sq_sb[:, g : g + 1],
                )
                nc.sync.dma_start(
                    out=hpreT[:, g * P : g * P + P], in_=hT_sb[:]
                )

            stat_sb = cp.tile([F, 2], dt.float32)
            nc.vector.reduce_sum(
                out=stat_sb[:, 0:1], in_=sum_sb[:], axis=mybir.AxisListType.X
            )
            nc.vector.reduce_sum(
                out=stat_sb[:, 1:2], in_=sq_sb[:], axis=mybir.AxisListType.X
            )
            nc.sync.dma_start(out=stats[:], in_=stat_sb[:])

    nc.compile()
    nc_cache["agg"] = nc
    return nc


def build_transform(readout, nc_cache={}):
    """Transform launch: global BN stats -> relu(a*h+c).

    readout=False: output hpost [NPAD, 64] row-major (for next gather).
    readout=True:  output y [1, 2] partial logits (sum_own relu(...) @ Wc).

    Inputs per core:
      hT   [64, NPAD] f32   own pre-BN h (transposed)
      sall [64, 16]  f32    8 cores' [sum, sumsq] partials, interleaved
      gb   [64, 2]   f32    gamma, beta
      Wc   [64, 2]   f32    classifier weight (readout only; else ignored)
    """
    key = ("tr", readout)
    if key in nc_cache:
        return nc_cache[key]
    nc = bacc.Bacc("TRN2", target_bir_lowering=False, debug=False)
    hT = nc.dram_tensor("hT", [F, NPAD], dt.float32, kind="ExternalInput")
    sall = nc.dram_tensor("sall", [F, 2 * NCORES], dt.float32, kind="ExternalInput")
    gb = nc.dram_tensor("gb", [F, 2], dt.float32, kind="ExternalInput")
    Wc = nc.dram_tensor("Wc", [F, 2], dt.float32, kind="ExternalInput")
    padc = nc.dram_tensor("padc", [F, 1], dt.float32, kind="ExternalInput")
    if readout:
        yout = nc.dram_tensor("y", [1, 2], dt.float32, kind="ExternalOutput")
    else:
        hpost = nc.dram_tensor("hpost", [NPAD, F], dt.float32, kind="ExternalOutput")

    with tile.TileContext(nc) as tc:
        with (
            tc.tile_pool(name="cp", bufs=1) as cp,
            tc.tile_pool(name="ep", bufs=2) as ep,
            tc.tile_pool(name="pp", bufs=2, space="PSUM") as pp,
        ):
            hT_t = cp.tile([F, NPAD], dt.float32)
            nc.sync.dma_start(out=hT_t[:], in_=hT[:])
            sall_t = cp.tile([F, 2 * NCORES], dt.float32)
            nc.sync.dma_start(out=sall_t[:], in_=sall[:])
            gb_t = cp.tile([F, 2], dt.float32)
            nc.sync.dma_start(out=gb_t[:], in_=gb[:])
            Wc_t = cp.tile([F, 2], dt.float32)
            nc.sync.dma_start(out=Wc_t[:], in_=Wc[:])
            padc_t = cp.tile([F, 1], dt.float32)
            nc.sync.dma_start(out=padc_t[:], in_=padc[:])

            # stats: columns 0..7 sums, 8..15 sumsqs (host packs that way)
            scr = cp.tile([F, 8], dt.float32)
            nc.vector.reduce_sum(
                out=scr[:, 0:1], in_=sall_t[:, :NCORES], axis=mybir.AxisListType.X
            )
            nc.vector.reduce_sum(
                out=scr[:, 1:2], in_=sall_t[:, NCORES:], axis=mybir.AxisListType.X
            )
            inv_n = 1.0 / float(N)
            # mu = sum/N ; msq = sumsq/N ; var = msq - mu^2
            nc.vector.tensor_scalar(
                out=scr[:, 2:3], in0=scr[:, 0:1], scalar1=inv_n, scalar2=None,
                op0=mybir.AluOpType.mult,
            )  # mu
            nc.vector.tensor_scalar(
                out=scr[:, 3:4], in0=scr[:, 1:2], scalar1=inv_n, scalar2=None,
                op0=mybir.AluOpType.mult,
            )  # msq
            musq = cp.tile([F, 1], dt.float32)
            nc.vector.tensor_tensor(
                out=musq[:], in0=scr[:, 2:3], in1=scr[:, 2:3],
                op=mybir.AluOpType.mult,
            )
            var_eps = cp.tile([F, 1], dt.float32)
            nc.vector.tensor_tensor(
                out=var_eps[:], in0=scr[:, 3:4], in1=musq[:],
                op=mybir.AluOpType.subtract,
            )
            nc.vector.tensor_scalar(
                out=var_eps[:], in0=var_eps[:], scalar1=float(EPS), scalar2=None,
                op0=mybir.AluOpType.add,
            )
            std = cp.tile([F, 1], dt.float32)
            nc.scalar.activation(
                out=std[:], in_=var_eps[:], func=mybir.ActivationFunctionType.Sqrt
            )
            inv_std = cp.tile([F, 1], dt.float32)
            nc.vector.reciprocal(out=inv_std[:], in_=std[:])
            a_col = cp.tile([F, 1], dt.float32)
            nc.vector.tensor_tensor(
                out=a_col[:], in0=gb_t[:, 0:1], in1=inv_std[:],
                op=mybir.AluOpType.mult,
            )
            # c = beta - mu*a
            mua = cp.tile([F, 1], dt.float32)
            nc.vector.tensor_tensor(
                out=mua[:], in0=scr[:, 2:3], in1=a_col[:], op=mybir.AluOpType.mult
            )
            c_col = cp.tile([F, 1], dt.float32)
            nc.vector.tensor_tensor(
                out=c_col[:], in0=gb_t[:, 1:2], in1=mua[:],
                op=mybir.AluOpType.subtract,
            )

            hpostT = cp.tile([F, NPAD], dt.float32)
            nc.scalar.activation(
                out=hpostT[:],
                in_=hT_t[:],
                func=mybir.ActivationFunctionType.Relu,
                scale=a_col[:],
                bias=c_col[:],
            )

            if readout:
                # sum over all cols, then subtract pad_count * relu(c)
                acc = cp.tile([F, 1], dt.float32)
                nc.vector.reduce_sum(
                    out=acc[:], in_=hpostT[:], axis=mybir.AxisListType.X
                )
                relu_c = cp.tile([F, 1], dt.float32)
                nc.scalar.activation(
                    out=relu_c[:], in_=c_col[:],
                    func=mybir.ActivationFunctionType.Relu,
                )
                padsum = cp.tile([F, 1], dt.float32)
                nc.vector.tensor_tensor(
                    out=padsum[:], in0=relu_c[:], in1=padc_t[:],
                    op=mybir.AluOpType.mult,
                )
                nc.vector.tensor_tensor(
                    out=acc[:], in0=acc[:], in1=padsum[:],
                    op=mybir.AluOpType.subtract,
                )
                y_ps = pp.tile([1, 2], dt.float32, tag="y")
                nc.tensor.matmul(
                    out=y_ps[:], lhsT=acc[:], rhs=Wc_t[:], start=True, stop=True
                )
                y_sb = cp.tile([1, 2], dt.float32)
                nc.vector.tensor_copy(out=y_sb[:], in_=y_ps[:])
                nc.sync.dma_start(out=yout[:], in_=y_sb[:])
            else:
                ident = cp.tile([F, F], dt.float32)
                make_identity(nc, ident[:])
                for g in range(CH):
                    tr_ps = pp.tile([P, F], dt.float32, tag="tr")
                    nc.tensor.transpose(
                        out=tr_ps[:],
                        in_=hpostT[:, g * P : g * P + P],
                        identity=ident[:],
                    )
                    tr_sb = ep.tile([P, F], dt.float32, tag="trsb")
                    nc.vector.tensor_copy(out=tr_sb[:], in_=tr_ps[:])
                    nc.sync.dma_start(
                        out=hpost[g * P : g * P + P, :], in_=tr_sb[:]
                    )

    nc.compile()
    nc_cache[key] = nc
    return nc


# --------------------------------------------------------------------------
# Host-side orchestration
# --------------------------------------------------------------------------

def _prep_edges(src, dst):
    """Per-core edge arrays: idx/seg/w tiles [128, CH*T]."""
    deg_out = np.bincount(src, minlength=N).astype(np.float64)
    deg_in = np.bincount(dst, minlength=N).astype(np.float64)
    r_out = 1.0 / np.sqrt(np.maximum(deg_out, 1.0))
    r_in = 1.0 / np.sqrt(np.maximum(deg_in, 1.0))
    w_edge = (r_out[src] * r_in[dst]).astype(np.float32)

    chunk_of = dst // P  # global chunk id (0..781)
    order = np.lexsort((src, chunk_of))
    src_s = src[order]
    dst_s = dst[order]
    w_s = w_edge[order]
    chunk_s = chunk_of[order]

    counts = np.bincount(chunk_s, minlength=NCORES * CH)
    assert counts.max() <= T * P, f"chunk overflow: {counts.max()} > {T * P}"
    bounds = np.concatenate([[0], np.cumsum(counts)])

    per_core = []
    for c in range(NCORES):
        idx_a = np.zeros((CH * T * P,), np.int32)
        seg_a = np.full((CH * T * P,), SEG_PAD, np.float32)
        w_a = np.zeros((CH * T * P,), np.float32)
        for g in range(CH):
            gc = c * CH + g
            lo, hi = bounds[gc], bounds[gc + 1]
            n = hi - lo
            base = g * T * P
            idx_a[base : base + n] = src_s[lo:hi]
            seg_a[base : base + n] = (dst_s[lo:hi] - gc * P).astype(np.float32)
            w_a[base : base + n] = w_s[lo:hi]
        # lay out edge (t, p) -> tile[p, t]
        idx_tile = idx_a.reshape(CH * T, P).T.copy()
        seg_tile = seg_a.reshape(CH * T, P).T.copy()
        w_tile = w_a.reshape(CH * T, P).T.copy()
        per_core.append((idx_tile, seg_tile, w_tile))
    return per_core


def _pad_rows(x):
    NROWS = N + 352
    out = np.zeros((NROWS, F), np.float32)
    out[:N] = x
    return out


REAL = [min(NPAD, N - c * NPAD) for c in range(NCORES)]  # 12544 x7, 12192


def kernel(x, src, dst, W1, b1, g1, be1, W2, b2, g2, be2, Wc, bc):
    x = np.asarray(x, np.float32)
    src = np.asarray(src, np.int32)
    dst = np.asarray(dst, np.int32)
    per_core = _prep_edges(src, dst)

    agg = build_agg()
    tr_mid = build_transform(readout=False)
    tr_end = build_transform(readout=True)
    t_total = 0
    kernel.launch_times_ns = []

    def agg_layer(x_full, Wl):
        xin = _pad_rows(x_full)
        in_maps = []
        for c in range(NCORES):
            idx_t, seg_t, w_t = per_core[c]
            in_maps.append(
                {
                    "xin": xin,
                    "idx": idx_t,
                    "seg": seg_t,
                    "w": w_t,
                    "Wt": np.asarray(Wl, np.float32),
                }
            )
        return _run(agg, in_maps)

    def transform_maps(res_agg, gl, bel, Wc_):
        st = [r["stats"] for r in res_agg.results]
        sall = np.concatenate(
            [np.stack([s[:, 0] for s in st], 1), np.stack([s[:, 1] for s in st], 1)],
            axis=1,
        ).astype(np.float32)
        gbv = np.stack(
            [np.asarray(gl, np.float32), np.asarray(bel, np.float32)], axis=1
        )
        Wcv = np.asarray(Wc_, np.float32)
        return [
            {
                "hT": res_agg.results[c]["hpreT"],
                "sall": sall,
                "gb": gbv,
                "Wc": Wcv,
                "padc": np.full((F, 1), float(NPAD - REAL[c]), np.float32),
            }
            for c in range(NCORES)
        ]

    zero_wc = np.zeros((F, 2), np.float32)

    r1 = agg_layer(x, W1)
    t_total += r1.exec_time_ns or 0
    kernel.launch_times_ns.append(r1.exec_time_ns)
    r2 = _run(tr_mid, transform_maps(r1, g1, be1, zero_wc))
    t_total += r2.exec_time_ns or 0
    kernel.launch_times_ns.append(r2.exec_time_ns)
    h1_full = np.concatenate(
        [r2.results[c]["hpost"][: REAL[c]] for c in range(NCORES)], axis=0
    )
    r3 = agg_layer(h1_full, W2)
    t_total += r3.exec_time_ns or 0
    kernel.launch_times_ns.append(r3.exec_time_ns)
    r4 = _run(tr_end, transform_maps(r3, g2, be2, Wc))
    t_total += r4.exec_time_ns or 0
    kernel.launch_times_ns.append(r4.exec_time_ns)

    y = sum(np.asarray(r4.results[c]["y"], np.float64) for c in range(NCORES))
    out = (y / float(N) + np.asarray(bc, np.float64)).astype(np.float32)
    kernel.last_exec_time_ns = t_total
    return out


# revision 6
# speedup vs baseline: 1.1855x; 1.1855x over previous
"""GraphConv x2 + BN + ReLU + mean-pool + classifier on 8 TRN2 cores.

Strategy (dst-sharded nodes, segment-sum as one-hot matmul):
  - Nodes are split into 8 contiguous blocks of 12500 (padded to 12544 =
    98 chunks x 128).  Each core owns the edges whose dst falls in its block
    (edge-cut partitioning by dst).
  - Edges per core are grouped by 128-node dst-chunk, sorted by src inside
    the chunk, padded per-chunk to T=18 subchunks of 128 edges.
  - Aggregation m^T[feat, seg] += G^T S per 128-edge subchunk:
      G   [128 edges, 64] gathered rows of the (replicated) feature table
      S   [128 edges, 128 segs] one-hot built on DVE from iota==seg, scaled
          by w_e = rsqrt(deg_out[src]) * rsqrt(deg_in[dst])  (norm='both')
    so PSUM accumulates the normalized message sum transposed.
  - Per chunk: h^T = W^T m^T via a second matmul (the conv bias is
    dropped: BatchNorm right after is shift-invariant); BN partial sums;
    h^T written to HBM (pre-BN).
  - BatchNorm needs global stats -> separate transform launch per layer:
    reduces the 8 cores' partials, applies relu(a*h + c), transposes to
    row-major for the next layer's gather (or mean-pool + classifier at
    the end).
  - Host work between launches is routing only (concat / slicing);
    final output = sum of per-core partial logits / N + bc.

Launches: L1 agg(x, W1) -> L2 transform1 -> L3 agg(h1, W2) -> L4
transform2+readout.  Conv biases b1/b2 cancel inside BatchNorm; bc is added
on the host along with the cross-core logit sum (pure routing + 2 adds).
"""
import sys

import numpy as np

sys.path.insert(0, "/opt/trn_rl_repo")

import concourse.bacc as bacc
import concourse.mybir as mybir
import concourse.tile as tile
from concourse.bass import IndirectOffsetOnAxis
from concourse.masks import make_identity

dt = mybir.dt

# ---- problem constants (fixed by the harness) ----
N = 100_000
E = 1_600_000
F = 64
NCORES = 8
P = 128
NPC = 12_500          # nodes per core
CH = 98               # 128-node chunks per core (98*128 = 12544)
NPAD = CH * P         # padded nodes per core
T = 18                # subchunks (of 128 edges) per chunk
EPS = 1e-5
SEG_PAD = 10_000.0    # seg id for pad edges (never matches iota 0..127)

_trace = {"on": False}


def _run(nc, in_maps, trace=None):
    from concourse.bass_utils import run_bass_kernel_spmd

    use_trace = _trace["on"] if trace is None else trace
    if use_trace:
        try:
            import ntff_hook

            ntff_hook.install()
        except Exception:
            use_trace = False
    res = run_bass_kernel_spmd(
        nc,
        in_maps,
        list(range(NCORES)),
        trace=use_trace,
        trace_cores=[0] if use_trace else None,
    )
    return res


# --------------------------------------------------------------------------
# Launch builders
# --------------------------------------------------------------------------

def build_agg(nc_cache={}):
    """Aggregation launch: gather + segment-matmul + W matmul + stat partials.

    Inputs per core:
      xin  [N_ROWS, 64] f32   feature table (replicated, padded rows)
      idx  [128, CH*T] i32    src id of edge (subchunk t, lane p)
      seg  [128, CH*T] f32    dst-local seg id (0..127) or SEG_PAD
      w    [128, CH*T] f32    edge weight (0 for pad)
      Wt   [64, 64]  f32      layer weight
    Outputs:
      hpreT [64, NPAD] f32    pre-BN h, transposed (channels on partitions)
      stats [64, 2]   f32     [sum, sumsq] over this core's nodes
                              (pad columns are exact zeros)
    """
    if "agg" in nc_cache:
        return nc_cache["agg"]
    NROWS = N + 352  # 100352, multiple of 128
    nc = bacc.Bacc("TRN2", target_bir_lowering=False, debug=False)
    xin = nc.dram_tensor("xin", [NROWS, F], dt.float32, kind="ExternalInput")
    idx = nc.dram_tensor("idx", [P, CH * T], dt.int32, kind="ExternalInput")
    seg = nc.dram_tensor("seg", [P, CH * T], dt.float32, kind="ExternalInput")
    w = nc.dram_tensor("w", [P, CH * T], dt.float32, kind="ExternalInput")
    Wt = nc.dram_tensor("Wt", [F, F], dt.float32, kind="ExternalInput")
    hpreT = nc.dram_tensor("hpreT", [F, NPAD], dt.float32, kind="ExternalOutput")
    stats = nc.dram_tensor("stats", [F, 2], dt.float32, kind="ExternalOutput")

    with tile.TileContext(nc) as tc:
        with (
            tc.tile_pool(name="cp", bufs=1) as cp,
            tc.tile_pool(name="gp", bufs=3) as gp,
            tc.tile_pool(name="sp", bufs=3) as sp,
            tc.tile_pool(name="ep", bufs=2) as ep,
            tc.tile_pool(name="pp", bufs=2, space="PSUM") as pp,
        ):
            idx_t = cp.tile([P, CH * T], dt.int32)
            nc.sync.dma_start(out=idx_t[:], in_=idx[:])
            seg_t = cp.tile([P, CH * T], dt.float32)
            nc.sync.dma_start(out=seg_t[:], in_=seg[:])
            w_t = cp.tile([P, CH * T], dt.float32)
            nc.sync.dma_start(out=w_t[:], in_=w[:])
            W_t = cp.tile([F, F], dt.float32)
            nc.sync.dma_start(out=W_t[:], in_=Wt[:])

            iota_i = cp.tile([P, P], dt.int32)
            nc.gpsimd.iota(iota_i[:], pattern=[[1, P]], base=0, channel_multiplier=0)
            iota_f = cp.tile([P, P], dt.float32)
            nc.vector.tensor_copy(out=iota_f[:], in_=iota_i[:])

            sum_sb = cp.tile([F, CH], dt.float32)
            sq_sb = cp.tile([F, CH], dt.float32)

            for g in range(CH):
                G = gp.tile([P, T, F], dt.float32, tag="G")
                for t in range(T):
                    nc.gpsimd.indirect_dma_start(
                        out=G[:, t, :],
                        out_offset=None,
                        in_=xin[:],
                        in_offset=IndirectOffsetOnAxis(
                            ap=idx_t[:, g * T + t : g * T + t + 1], axis=0
                        ),
                    )
                mT_ps = pp.tile([F, P], dt.float32, tag="mT")
                for t in range(T):
                    S = sp.tile([P, P], dt.float32, tag="S")
                    nc.vector.tensor_scalar(
                        out=S[:],
                        in0=iota_f[:],
                        scalar1=seg_t[:, g * T + t : g * T + t + 1],
                        scalar2=w_t[:, g * T + t : g * T + t + 1],
                        op0=mybir.AluOpType.is_equal,
                        op1=mybir.AluOpType.mult,
                    )
                    nc.tensor.matmul(
                        out=mT_ps[:],
                        lhsT=G[:, t, :],
                        rhs=S[:],
                        start=(t == 0),
                        stop=(t == T - 1),
                    )
                mT_sb = ep.tile([F, P], dt.float32, tag="mTsb")
                nc.vector.tensor_copy(out=mT_sb[:], in_=mT_ps[:])
                hT_ps = pp.tile([F, P], dt.float32, tag="hT")
                nc.tensor.matmul(
                    out=hT_ps[:], lhsT=W_t[:], rhs=mT_sb[:], start=True, stop=True
                )
                # h = W^T m  (conv bias is BN-shift-invariant: dropped).
                # Pad node columns are exactly zero, so stats need no mask.
                hT_sb = ep.tile([F, P], dt.float32, tag="hTsb")
                nc.vector.tensor_copy(out=hT_sb[:], in_=hT_ps[:])
                nc.vector.reduce_sum(
                    out=sum_sb[:, g : g + 1], in_=hT_sb[:],
                    axis=mybir.AxisListType.X,
                )
                sq_scr = ep.tile([F, P], dt.float32, tag="sq")
                nc.scalar.activation(
                    out=sq_scr[:],
                    in_=hT_sb[:],
                    func=mybir.ActivationFunctionType.Square,
                    accum_out=sq_sb[:, g : g + 1],
                )
                nc.sync.dma_start(
                    out=hpreT[:, g * P : g * P + P], in_=hT_sb[:]
                )

            stat_sb = cp.tile([F, 2], dt.float32)
            nc.vector.reduce_sum(
                out=stat_sb[:, 0:1], in_=sum_sb[:], axis=mybir.AxisListType.X
            )
            nc.vector.reduce_sum(
                out=stat_sb[:, 1:2], in_=sq_sb[:], axis=mybir.AxisListType.X
            )
            nc.sync.dma_start(out=stats[:], in_=stat_sb[:])

    nc.compile()
    nc_cache["agg"] = nc
    return nc


def build_transform(readout, nc_cache={}):
    """Transform launch: global BN stats -> relu(a*h+c).

    readout=False: output hpost [NPAD, 64] row-major (for next gather).
    readout=True:  output y [1, 2] partial logits (sum_own relu(...) @ Wc).

    Inputs per core:
      hT   [64, NPAD] f32   own pre-BN h (transposed)
      sall [64, 16]  f32    8 cores' [sum, sumsq] partials, interleaved
      gb   [64, 2]   f32    gamma, beta
      Wc   [64, 2]   f32    classifier weight (readout only; else ignored)
    """
    key = ("tr", readout)
    if key in nc_cache:
        return nc_cache[key]
    nc = bacc.Bacc("TRN2", target_bir_lowering=False, debug=False)
    hT = nc.dram_tensor("hT", [F, NPAD], dt.float32, kind="ExternalInput")
    sall = nc.dram_tensor("sall", [F, 2 * NCORES], dt.float32, kind="ExternalInput")
    gb = nc.dram_tensor("gb", [F, 2], dt.float32, kind="ExternalInput")
    Wc = nc.dram_tensor("Wc", [F, 2], dt.float32, kind="ExternalInput")
    padc = nc.dram_tensor("padc", [F, 1], dt.float32, kind="ExternalInput")
    if readout:
        yout = nc.dram_tensor("y", [1, 2], dt.float32, kind="ExternalOutput")
    else:
        hpost = nc.dram_tensor("hpost", [NPAD, F], dt.float32, kind="ExternalOutput")

    with tile.TileContext(nc) as tc:
        with (
            tc.tile_pool(name="cp", bufs=1) as cp,
            tc.tile_pool(name="ep", bufs=2) as ep,
            tc.tile_pool(name="pp", bufs=2, space="PSUM") as pp,
        ):
            hT_t = cp.tile([F, NPAD], dt.float32)
            nc.sync.dma_start(out=hT_t[:], in_=hT[:])
            sall_t = cp.tile([F, 2 * NCORES], dt.float32)
            nc.sync.dma_start(out=sall_t[:], in_=sall[:])
            gb_t = cp.tile([F, 2], dt.float32)
            nc.sync.dma_start(out=gb_t[:], in_=gb[:])
            Wc_t = cp.tile([F, 2], dt.float32)
            nc.sync.dma_start(out=Wc_t[:], in_=Wc[:])
            padc_t = cp.tile([F, 1], dt.float32)
            nc.sync.dma_start(out=padc_t[:], in_=padc[:])

            # stats: columns 0..7 sums, 8..15 sumsqs (host packs that way)
            scr = cp.tile([F, 8], dt.float32)
            nc.vector.reduce_sum(
                out=scr[:, 0:1], in_=sall_t[:, :NCORES], axis=mybir.AxisListType.X
            )
            nc.vector.reduce_sum(
                out=scr[:, 1:2], in_=sall_t[:, NCORES:], axis=mybir.AxisListType.X
            )
            inv_n = 1.0 / float(N)
            # mu = sum/N ; msq = sumsq/N ; var = msq - mu^2
            nc.vector.tensor_scalar(
                out=scr[:, 2:3], in0=scr[:, 0:1], scalar1=inv_n, scalar2=None,
                op0=mybir.AluOpType.mult,
            )  # mu
            nc.vector.tensor_scalar(
                out=scr[:, 3:4], in0=scr[:, 1:2], scalar1=inv_n, scalar2=None,
                op0=mybir.AluOpType.mult,
            )  # msq
            musq = cp.tile([F, 1], dt.float32)
            nc.vector.tensor_tensor(
                out=musq[:], in0=scr[:, 2:3], in1=scr[:, 2:3],
                op=mybir.AluOpType.mult,
            )
            var_eps = cp.tile([F, 1], dt.float32)
            nc.vector.tensor_tensor(
                out=var_eps[:], in0=scr[:, 3:4], in1=musq[:],
                op=mybir.AluOpType.subtract,
            )
            nc.vector.tensor_scalar(
                out=var_eps[:], in0=var_eps[:], scalar1=float(EPS), scalar2=None,
                op0=mybir.AluOpType.add,
            )
            std = cp.tile([F, 1], dt.float32)
            nc.scalar.activation(
                out=std[:], in_=var_eps[:], func=mybir.ActivationFunctionType.Sqrt
            )
            inv_std = cp.tile([F, 1], dt.float32)
            nc.vector.reciprocal(out=inv_std[:], in_=std[:])
            a_col = cp.tile([F, 1], dt.float32)
            nc.vector.tensor_tensor(
                out=a_col[:], in0=gb_t[:, 0:1], in1=inv_std[:],
                op=mybir.AluOpType.mult,
            )
            # c = beta - mu*a
            mua = cp.tile([F, 1], dt.float32)
            nc.vector.tensor_tensor(
                out=mua[:], in0=scr[:, 2:3], in1=a_col[:], op=mybir.AluOpType.mult
            )
            c_col = cp.tile([F, 1], dt.float32)
            nc.vector.tensor_tensor(
                out=c_col[:], in0=gb_t[:, 1:2], in1=mua[:],
                op=mybir.AluOpType.subtract,
            )

            hpostT = cp.tile([F, NPAD], dt.float32)
            nc.scalar.activation(
                out=hpostT[:],
                in_=hT_t[:],
                func=mybir.ActivationFunctionType.Relu,
                scale=a_col[:],
                bias=c_col[:],
            )

            if readout:
                # sum over all cols, then subtract pad_count * relu(c)
                acc = cp.tile([F, 1], dt.float32)
                nc.vector.reduce_sum(
                    out=acc[:], in_=hpostT[:], axis=mybir.AxisListType.X
                )
                relu_c = cp.tile([F, 1], dt.float32)
                nc.scalar.activation(
                    out=relu_c[:], in_=c_col[:],
                    func=mybir.ActivationFunctionType.Relu,
                )
                padsum = cp.tile([F, 1], dt.float32)
                nc.vector.tensor_tensor(
                    out=padsum[:], in0=relu_c[:], in1=padc_t[:],
                    op=mybir.AluOpType.mult,
                )
                nc.vector.tensor_tensor(
                    out=acc[:], in0=acc[:], in1=padsum[:],
                    op=mybir.AluOpType.subtract,
                )
                y_ps = pp.tile([1, 2], dt.float32, tag="y")
                nc.tensor.matmul(
                    out=y_ps[:], lhsT=acc[:], rhs=Wc_t[:], start=True, stop=True
                )
                y_sb = cp.tile([1, 2], dt.float32)
                nc.vector.tensor_copy(out=y_sb[:], in_=y_ps[:])
                nc.sync.dma_start(out=yout[:], in_=y_sb[:])
            else:
                ident = cp.tile([F, F], dt.float32)
                make_identity(nc, ident[:])
                for g in range(CH):
                    tr_ps = pp.tile([P, F], dt.float32, tag="tr")
                    nc.tensor.transpose(
                        out=tr_ps[:],
                        in_=hpostT[:, g * P : g * P + P],
                        identity=ident[:],
                    )
                    tr_sb = ep.tile([P, F], dt.float32, tag="trsb")
                    nc.vector.tensor_copy(out=tr_sb[:], in_=tr_ps[:])
                    nc.sync.dma_start(
                        out=hpost[g * P : g * P + P, :], in_=tr_sb[:]
                    )

    nc.compile()
    nc_cache[key] = nc
    return nc


# --------------------------------------------------------------------------
# Host-side orchestration
# --------------------------------------------------------------------------

def _prep_edges(src, dst):
    """Per-core edge arrays: idx/seg/w tiles [128, CH*T]."""
    deg_out = np.bincount(src, minlength=N).astype(np.float64)
    deg_in = np.bincount(dst, minlength=N).astype(np.float64)
    r_out = 1.0 / np.sqrt(np.maximum(deg_out, 1.0))
    r_in = 1.0 / np.sqrt(np.maximum(deg_in, 1.0))
    w_edge = (r_out[src] * r_in[dst]).astype(np.float32)

    chunk_of = dst // P  # global chunk id (0..781)
    order = np.lexsort((src, chunk_of))
    src_s = src[order]
    dst_s = dst[order]
    w_s = w_edge[order]
    chunk_s = chunk_of[order]

    counts = np.bincount(chunk_s, minlength=NCORES * CH)
    assert counts.max() <= T * P, f"chunk overflow: {counts.max()} > {T * P}"
    bounds = np.concatenate([[0], np.cumsum(counts)])

    per_core = []
    for c in range(NCORES):
        idx_a = np.zeros((CH * T * P,), np.int32)
        seg_a = np.full((CH * T * P,), SEG_PAD, np.float32)
        w_a = np.zeros((CH * T * P,), np.float32)
        for g in range(CH):
            gc = c * CH + g
            lo, hi = bounds[gc], bounds[gc + 1]
            n = hi - lo
            base = g * T * P
            idx_a[base : base + n] = src_s[lo:hi]
            seg_a[base : base + n] = (dst_s[lo:hi] - gc * P).astype(np.float32)
            w_a[base : base + n] = w_s[lo:hi]
        # lay out edge (t, p) -> tile[p, t]
        idx_tile = idx_a.reshape(CH * T, P).T.copy()
        seg_tile = seg_a.reshape(CH * T, P).T.copy()
        w_tile = w_a.reshape(CH * T, P).T.copy()
        per_core.append((idx_tile, seg_tile, w_tile))
    return per_core


def _pad_rows(x):
    NROWS = N + 352
    out = np.zeros((NROWS, F), np.float32)
    out[:N] = x
    return out


REAL = [min(NPAD, N - c * NPAD) for c in range(NCORES)]  # 12544 x7, 12192


def kernel(x, src, dst, W1, b1, g1, be1, W2, b2, g2, be2, Wc, bc):
    x = np.asarray(x, np.float32)
    src = np.asarray(src, np.int32)
    dst = np.asarray(dst, np.int32)
    per_core = _prep_edges(src, dst)

    agg = build_agg()
    tr_mid = build_transform(readout=False)
    tr_end = build_transform(readout=True)
    t_total = 0
    kernel.launch_times_ns = []

    def agg_layer(x_full, Wl):
        xin = _pad_rows(x_full)
        in_maps = []
        for c in range(NCORES):
            idx_t, seg_t, w_t = per_core[c]
            in_maps.append(
                {
                    "xin": xin,
                    "idx": idx_t,
                    "seg": seg_t,
                    "w": w_t,
                    "Wt": np.asarray(Wl, np.float32),
                }
            )
        return _run(agg, in_maps)

    def transform_maps(res_agg, gl, bel, Wc_):
        st = [r["stats"] for r in res_agg.results]
        sall = np.concatenate(
            [np.stack([s[:, 0] for s in st], 1), np.stack([s[:, 1] for s in st], 1)],
            axis=1,
        ).astype(np.float32)
        gbv = np.stack(
            [np.asarray(gl, np.float32), np.asarray(bel, np.float32)], axis=1
        )
        Wcv = np.asarray(Wc_, np.float32)
        return [
            {
                "hT": res_agg.results[c]["hpreT"],
                "sall": sall,
                "gb": gbv,
                "Wc": Wcv,
                "padc": np.full((F, 1), float(NPAD - REAL[c]), np.float32),
            }
            for c in range(NCORES)
        ]

    zero_wc = np.zeros((F, 2), np.float32)

    r1 = agg_layer(x, W1)
    t_total += r1.exec_time_ns or 0
    kernel.launch_times_ns.append(r1.exec_time_ns)
    r2 = _run(tr_mid, transform_maps(r1, g1, be1, zero_wc))
    t_total += r2.exec_time_ns or 0
    kernel.launch_times_ns.append(r2.exec_time_ns)
    h1_full = np.concatenate(
        [r2.results[c]["hpost"][: REAL[c]] for c in range(NCORES)], axis=0
    )
    r3 = agg_layer(h1_full, W2)
    t_total += r3.exec_time_ns or 0
    kernel.launch_times_ns.append(r3.exec_time_ns)
    r4 = _run(tr_end, transform_maps(r3, g2, be2, Wc))
    t_total += r4.exec_time_ns or 0
    kernel.launch_times_ns.append(r4.exec_time_ns)

    y = sum(np.asarray(r4.results[c]["y"], np.float64) for c in range(NCORES))
    out = (y / float(N) + np.asarray(bc, np.float64)).astype(np.float32)
    kernel.last_exec_time_ns = t_total
    return out


# revision 7
# speedup vs baseline: 1.1946x; 1.0077x over previous
"""GraphConv x2 + BN + ReLU + mean-pool + classifier on 8 TRN2 cores.

Strategy (dst-sharded nodes, segment-sum as one-hot matmul):
  - Nodes are split into 8 contiguous blocks of 12500 (padded to 12544 =
    98 chunks x 128).  Each core owns the edges whose dst falls in its block
    (edge-cut partitioning by dst).
  - Edges per core are grouped by 128-node dst-chunk, sorted by src inside
    the chunk, padded per-chunk to T=18 subchunks of 128 edges.
  - Aggregation m^T[feat, seg] += G^T S per 128-edge subchunk:
      G   [128 edges, 64] gathered rows of the (replicated) feature table
      S   [128 edges, 128 segs] one-hot built on DVE from iota==seg, scaled
          by w_e = rsqrt(deg_out[src]) * rsqrt(deg_in[dst])  (norm='both')
    so PSUM accumulates the normalized message sum transposed.
  - Per chunk: h^T = W^T m^T via a second matmul (the conv bias is
    dropped: BatchNorm right after is shift-invariant); BN partial sums;
    h^T written to HBM (pre-BN).
  - BatchNorm needs global stats -> separate transform launch per layer:
    reduces the 8 cores' partials, applies relu(a*h + c), transposes to
    row-major for the next layer's gather (or mean-pool + classifier at
    the end).
  - Host work between launches is routing only (concat / slicing);
    final output = sum of per-core partial logits / N + bc.

Launches: L1 agg(x, W1) -> L2 transform1 -> L3 agg(h1, W2) -> L4
transform2+readout.  Conv biases b1/b2 cancel inside BatchNorm; bc is added
on the host along with the cross-core logit sum (pure routing + 2 adds).
"""
import sys

import numpy as np

sys.path.insert(0, "/opt/trn_rl_repo")

import concourse.bacc as bacc
import concourse.mybir as mybir
import concourse.tile as tile
from concourse.bass import IndirectOffsetOnAxis
from concourse.masks import make_identity

dt = mybir.dt

# ---- problem constants (fixed by the harness) ----
N = 100_000
E = 1_600_000
F = 64
NCORES = 8
P = 128
NPC = 12_500          # nodes per core
CH = 98               # 128-node chunks per core (98*128 = 12544)
NPAD = CH * P         # padded nodes per core
T = 18                # subchunks (of 128 edges) per chunk
EPS = 1e-5
SEG_PAD = 10_000.0    # seg id for pad edges (never matches iota 0..127)

_trace = {"on": False}


def _run(nc, in_maps, trace=None):
    from concourse.bass_utils import run_bass_kernel_spmd

    use_trace = _trace["on"] if trace is None else trace
    if use_trace:
        try:
            import ntff_hook

            ntff_hook.install()
        except Exception:
            use_trace = False
    res = run_bass_kernel_spmd(
        nc,
        in_maps,
        list(range(NCORES)),
        trace=use_trace,
        trace_cores=[0] if use_trace else None,
    )
    return res


# --------------------------------------------------------------------------
# Launch builders
# --------------------------------------------------------------------------

def build_agg(nc_cache={}):
    """Aggregation launch: gather + segment-matmul + W matmul + stat partials.

    Inputs per core:
      xin  [N_ROWS, 64] f32   feature table (replicated, padded rows)
      idx  [128, CH*T] i32    src id of edge (subchunk t, lane p)
      seg  [128, CH*T] f32    dst-local seg id (0..127) or SEG_PAD
      w    [128, CH*T] f32    edge weight (0 for pad)
      Wt   [64, 64]  f32      layer weight
    Outputs:
      hpreT [64, NPAD] f32    pre-BN h, transposed (channels on partitions)
      stats [64, 2]   f32     [sum, sumsq] over this core's nodes
                              (pad columns are exact zeros)
    """
    if "agg" in nc_cache:
        return nc_cache["agg"]
    NROWS = N + 352  # 100352, multiple of 128
    nc = bacc.Bacc("TRN2", target_bir_lowering=False, debug=False)
    xin = nc.dram_tensor("xin", [NROWS, F], dt.float32, kind="ExternalInput")
    idx = nc.dram_tensor("idx", [P, CH * T], dt.int32, kind="ExternalInput")
    seg = nc.dram_tensor("seg", [P, CH * T], dt.float32, kind="ExternalInput")
    w = nc.dram_tensor("w", [P, CH * T], dt.float32, kind="ExternalInput")
    Wt = nc.dram_tensor("Wt", [F, F], dt.float32, kind="ExternalInput")
    hpreT = nc.dram_tensor("hpreT", [F, NPAD], dt.float32, kind="ExternalOutput")
    stats = nc.dram_tensor("stats", [F, 2], dt.float32, kind="ExternalOutput")

    with tile.TileContext(nc) as tc:
        with (
            tc.tile_pool(name="cp", bufs=1) as cp,
            tc.tile_pool(name="gp", bufs=6) as gp,
            tc.tile_pool(name="sp", bufs=4) as sp,
            tc.tile_pool(name="ep", bufs=2) as ep,
            tc.tile_pool(name="pp", bufs=3, space="PSUM") as pp,
        ):
            idx_t = cp.tile([P, CH * T], dt.int32)
            nc.sync.dma_start(out=idx_t[:], in_=idx[:])
            seg_t = cp.tile([P, CH * T], dt.float32)
            nc.sync.dma_start(out=seg_t[:], in_=seg[:])
            w_t = cp.tile([P, CH * T], dt.float32)
            nc.sync.dma_start(out=w_t[:], in_=w[:])
            W_t = cp.tile([F, F], dt.float32)
            nc.sync.dma_start(out=W_t[:], in_=Wt[:])

            iota_i = cp.tile([P, P], dt.int32)
            nc.gpsimd.iota(iota_i[:], pattern=[[1, P]], base=0, channel_multiplier=0)
            iota_f = cp.tile([P, P], dt.float32)
            nc.vector.tensor_copy(out=iota_f[:], in_=iota_i[:])

            sum_sb = cp.tile([F, CH], dt.float32)
            sq_sb = cp.tile([F, CH], dt.float32)

            for g in range(CH):
                G = gp.tile([P, T, F], dt.float32, tag="G")
                for t in range(T):
                    nc.gpsimd.indirect_dma_start(
                        out=G[:, t, :],
                        out_offset=None,
                        in_=xin[:],
                        in_offset=IndirectOffsetOnAxis(
                            ap=idx_t[:, g * T + t : g * T + t + 1], axis=0
                        ),
                    )
                mT_ps = pp.tile([F, P], dt.float32, tag="mT")
                for t in range(T):
                    S = sp.tile([P, P], dt.float32, tag="S")
                    nc.vector.tensor_scalar(
                        out=S[:],
                        in0=iota_f[:],
                        scalar1=seg_t[:, g * T + t : g * T + t + 1],
                        scalar2=w_t[:, g * T + t : g * T + t + 1],
                        op0=mybir.AluOpType.is_equal,
                        op1=mybir.AluOpType.mult,
                    )
                    nc.tensor.matmul(
                        out=mT_ps[:],
                        lhsT=G[:, t, :],
                        rhs=S[:],
                        start=(t == 0),
                        stop=(t == T - 1),
                    )
                mT_sb = ep.tile([F, P], dt.float32, tag="mTsb")
                nc.vector.tensor_copy(out=mT_sb[:], in_=mT_ps[:])
                hT_ps = pp.tile([F, P], dt.float32, tag="hT")
                nc.tensor.matmul(
                    out=hT_ps[:], lhsT=W_t[:], rhs=mT_sb[:], start=True, stop=True
                )
                # h = W^T m  (conv bias is BN-shift-invariant: dropped).
                # Pad node columns are exactly zero, so stats need no mask.
                hT_sb = ep.tile([F, P], dt.float32, tag="hTsb")
                nc.vector.tensor_copy(out=hT_sb[:], in_=hT_ps[:])
                nc.vector.reduce_sum(
                    out=sum_sb[:, g : g + 1], in_=hT_sb[:],
                    axis=mybir.AxisListType.X,
                )
                sq_scr = ep.tile([F, P], dt.float32, tag="sq")
                nc.scalar.activation(
                    out=sq_scr[:],
                    in_=hT_sb[:],
                    func=mybir.ActivationFunctionType.Square,
                    accum_out=sq_sb[:, g : g + 1],
                )
                nc.sync.dma_start(
                    out=hpreT[:, g * P : g * P + P], in_=hT_sb[:]
                )

            stat_sb = cp.tile([F, 2], dt.float32)
            nc.vector.reduce_sum(
                out=stat_sb[:, 0:1], in_=sum_sb[:], axis=mybir.AxisListType.X
            )
            nc.vector.reduce_sum(
                out=stat_sb[:, 1:2], in_=sq_sb[:], axis=mybir.AxisListType.X
            )
            nc.sync.dma_start(out=stats[:], in_=stat_sb[:])

    nc.compile()
    nc_cache["agg"] = nc
    return nc


def build_transform(readout, nc_cache={}):
    """Transform launch: global BN stats -> relu(a*h+c).

    readout=False: output hpost [NPAD, 64] row-major (for next gather).
    readout=True:  output y [1, 2] partial logits (sum_own relu(...) @ Wc).

    Inputs per core:
      hT   [64, NPAD] f32   own pre-BN h (transposed)
      sall [64, 16]  f32    8 cores' [sum, sumsq] partials, interleaved
      gb   [64, 2]   f32    gamma, beta
      Wc   [64, 2]   f32    classifier weight (readout only; else ignored)
    """
    key = ("tr", readout)
    if key in nc_cache:
        return nc_cache[key]
    nc = bacc.Bacc("TRN2", target_bir_lowering=False, debug=False)
    hT = nc.dram_tensor("hT", [F, NPAD], dt.float32, kind="ExternalInput")
    sall = nc.dram_tensor("sall", [F, 2 * NCORES], dt.float32, kind="ExternalInput")
    gb = nc.dram_tensor("gb", [F, 2], dt.float32, kind="ExternalInput")
    Wc = nc.dram_tensor("Wc", [F, 2], dt.float32, kind="ExternalInput")
    padc = nc.dram_tensor("padc", [F, 1], dt.float32, kind="ExternalInput")
    if readout:
        yout = nc.dram_tensor("y", [1, 2], dt.float32, kind="ExternalOutput")
    else:
        hpost = nc.dram_tensor("hpost", [NPAD, F], dt.float32, kind="ExternalOutput")

    with tile.TileContext(nc) as tc:
        with (
            tc.tile_pool(name="cp", bufs=1) as cp,
            tc.tile_pool(name="ep", bufs=2) as ep,
            tc.tile_pool(name="pp", bufs=2, space="PSUM") as pp,
        ):
            hT_t = cp.tile([F, NPAD], dt.float32)
            nc.sync.dma_start(out=hT_t[:], in_=hT[:])
            sall_t = cp.tile([F, 2 * NCORES], dt.float32)
            nc.sync.dma_start(out=sall_t[:], in_=sall[:])
            gb_t = cp.tile([F, 2], dt.float32)
            nc.sync.dma_start(out=gb_t[:], in_=gb[:])
            Wc_t = cp.tile([F, 2], dt.float32)
            nc.sync.dma_start(out=Wc_t[:], in_=Wc[:])
            padc_t = cp.tile([F, 1], dt.float32)
            nc.sync.dma_start(out=padc_t[:], in_=padc[:])

            # stats: columns 0..7 sums, 8..15 sumsqs (host packs that way)
            scr = cp.tile([F, 8], dt.float32)
            nc.vector.reduce_sum(
                out=scr[:, 0:1], in_=sall_t[:, :NCORES], axis=mybir.AxisListType.X
            )
            nc.vector.reduce_sum(
                out=scr[:, 1:2], in_=sall_t[:, NCORES:], axis=mybir.AxisListType.X
            )
            inv_n = 1.0 / float(N)
            # mu = sum/N ; msq = sumsq/N ; var = msq - mu^2
            nc.vector.tensor_scalar(
                out=scr[:, 2:3], in0=scr[:, 0:1], scalar1=inv_n, scalar2=None,
                op0=mybir.AluOpType.mult,
            )  # mu
            nc.vector.tensor_scalar(
                out=scr[:, 3:4], in0=scr[:, 1:2], scalar1=inv_n, scalar2=None,
                op0=mybir.AluOpType.mult,
            )  # msq
            musq = cp.tile([F, 1], dt.float32)
            nc.vector.tensor_tensor(
                out=musq[:], in0=scr[:, 2:3], in1=scr[:, 2:3],
                op=mybir.AluOpType.mult,
            )
            var_eps = cp.tile([F, 1], dt.float32)
            nc.vector.tensor_tensor(
                out=var_eps[:], in0=scr[:, 3:4], in1=musq[:],
                op=mybir.AluOpType.subtract,
            )
            nc.vector.tensor_scalar(
                out=var_eps[:], in0=var_eps[:], scalar1=float(EPS), scalar2=None,
                op0=mybir.AluOpType.add,
            )
            std = cp.tile([F, 1], dt.float32)
            nc.scalar.activation(
                out=std[:], in_=var_eps[:], func=mybir.ActivationFunctionType.Sqrt
            )
            inv_std = cp.tile([F, 1], dt.float32)
            nc.vector.reciprocal(out=inv_std[:], in_=std[:])
            a_col = cp.tile([F, 1], dt.float32)
            nc.vector.tensor_tensor(
                out=a_col[:], in0=gb_t[:, 0:1], in1=inv_std[:],
                op=mybir.AluOpType.mult,
            )
            # c = beta - mu*a
            mua = cp.tile([F, 1], dt.float32)
            nc.vector.tensor_tensor(
                out=mua[:], in0=scr[:, 2:3], in1=a_col[:], op=mybir.AluOpType.mult
            )
            c_col = cp.tile([F, 1], dt.float32)
            nc.vector.tensor_tensor(
                out=c_col[:], in0=gb_t[:, 1:2], in1=mua[:],
                op=mybir.AluOpType.subtract,
            )

            hpostT = cp.tile([F, NPAD], dt.float32)
            nc.scalar.activation(
                out=hpostT[:],
                in_=hT_t[:],
                func=mybir.ActivationFunctionType.Relu,
                scale=a_col[:],
                bias=c_col[:],
            )

            if readout:
                # sum over all cols, then subtract pad_count * relu(c)
                acc = cp.tile([F, 1], dt.float32)
                nc.vector.reduce_sum(
                    out=acc[:], in_=hpostT[:], axis=mybir.AxisListType.X
                )
                relu_c = cp.tile([F, 1], dt.float32)
                nc.scalar.activation(
                    out=relu_c[:], in_=c_col[:],
                    func=mybir.ActivationFunctionType.Relu,
                )
                padsum = cp.tile([F, 1], dt.float32)
                nc.vector.tensor_tensor(
                    out=padsum[:], in0=relu_c[:], in1=padc_t[:],
                    op=mybir.AluOpType.mult,
                )
                nc.vector.tensor_tensor(
                    out=acc[:], in0=acc[:], in1=padsum[:],
                    op=mybir.AluOpType.subtract,
                )
                y_ps = pp.tile([1, 2], dt.float32, tag="y")
                nc.tensor.matmul(
                    out=y_ps[:], lhsT=acc[:], rhs=Wc_t[:], start=True, stop=True
                )
                y_sb = cp.tile([1, 2], dt.float32)
                nc.vector.tensor_copy(out=y_sb[:], in_=y_ps[:])
                nc.sync.dma_start(out=yout[:], in_=y_sb[:])
            else:
                ident = cp.tile([F, F], dt.float32)
                make_identity(nc, ident[:])
                GRP = 7  # chunks per output DMA batch (98 = 14*7)
                for b in range(CH // GRP):
                    tr_sb = ep.tile([P, GRP, F], dt.float32, tag="trsb")
                    for j in range(GRP):
                        g = b * GRP + j
                        tr_ps = pp.tile([P, F], dt.float32, tag="tr")
                        nc.tensor.transpose(
                            out=tr_ps[:],
                            in_=hpostT[:, g * P : g * P + P],
                            identity=ident[:],
                        )
                        nc.vector.tensor_copy(out=tr_sb[:, j, :], in_=tr_ps[:])
                    nc.sync.dma_start(
                        out=hpost[b * GRP * P : (b + 1) * GRP * P, :].rearrange(
                            "(p j) f -> p j f", j=GRP
                        ),
                        in_=tr_sb[:],
                    )

    nc.compile()
    nc_cache[key] = nc
    return nc


# --------------------------------------------------------------------------
# Host-side orchestration
# --------------------------------------------------------------------------

def _prep_edges(src, dst):
    """Per-core edge arrays: idx/seg/w tiles [128, CH*T]."""
    deg_out = np.bincount(src, minlength=N).astype(np.float64)
    deg_in = np.bincount(dst, minlength=N).astype(np.float64)
    r_out = 1.0 / np.sqrt(np.maximum(deg_out, 1.0))
    r_in = 1.0 / np.sqrt(np.maximum(deg_in, 1.0))
    w_edge = (r_out[src] * r_in[dst]).astype(np.float32)

    chunk_of = dst // P  # global chunk id (0..781)
    order = np.lexsort((src, chunk_of))
    src_s = src[order]
    dst_s = dst[order]
    w_s = w_edge[order]
    chunk_s = chunk_of[order]

    counts = np.bincount(chunk_s, minlength=NCORES * CH)
    assert counts.max() <= T * P, f"chunk overflow: {counts.max()} > {T * P}"
    bounds = np.concatenate([[0], np.cumsum(counts)])

    per_core = []
    for c in range(NCORES):
        idx_a = np.zeros((CH * T * P,), np.int32)
        seg_a = np.full((CH * T * P,), SEG_PAD, np.float32)
        w_a = np.zeros((CH * T * P,), np.float32)
        for g in range(CH):
            gc = c * CH + g
            lo, hi = bounds[gc], bounds[gc + 1]
            n = hi - lo
            base = g * T * P
            idx_a[base : base + n] = src_s[lo:hi]
            seg_a[base : base + n] = (dst_s[lo:hi] - gc * P).astype(np.float32)
            w_a[base : base + n] = w_s[lo:hi]
        # lay out edge (t, p) -> tile[p, t]
        idx_tile = idx_a.reshape(CH * T, P).T.copy()
        seg_tile = seg_a.reshape(CH * T, P).T.copy()
        w_tile = w_a.reshape(CH * T, P).T.copy()
        per_core.append((idx_tile, seg_tile, w_tile))
    return per_core


def _pad_rows(x):
    NROWS = N + 352
    out = np.zeros((NROWS, F), np.float32)
    out[:N] = x
    return out


REAL = [min(NPAD, N - c * NPAD) for c in range(NCORES)]  # 12544 x7, 12192


def kernel(x, src, dst, W1, b1, g1, be1, W2, b2, g2, be2, Wc, bc):
    x = np.asarray(x, np.float32)
    src = np.asarray(src, np.int32)
    dst = np.asarray(dst, np.int32)
    per_core = _prep_edges(src, dst)

    agg = build_agg()
    tr_mid = build_transform(readout=False)
    tr_end = build_transform(readout=True)
    t_total = 0
    kernel.launch_times_ns = []

    def agg_layer(x_full, Wl):
        xin = _pad_rows(x_full)
        in_maps = []
        for c in range(NCORES):
            idx_t, seg_t, w_t = per_core[c]
            in_maps.append(
                {
                    "xin": xin,
                    "idx": idx_t,
                    "seg": seg_t,
                    "w": w_t,
                    "Wt": np.asarray(Wl, np.float32),
                }
            )
        return _run(agg, in_maps)

    def transform_maps(res_agg, gl, bel, Wc_):
        st = [r["stats"] for r in res_agg.results]
        sall = np.concatenate(
            [np.stack([s[:, 0] for s in st], 1), np.stack([s[:, 1] for s in st], 1)],
            axis=1,
        ).astype(np.float32)
        gbv = np.stack(
            [np.asarray(gl, np.float32), np.asarray(bel, np.float32)], axis=1
        )
        Wcv = np.asarray(Wc_, np.float32)
        return [
            {
                "hT": res_agg.results[c]["hpreT"],
                "sall": sall,
                "gb": gbv,
                "Wc": Wcv,
                "padc": np.full((F, 1), float(NPAD - REAL[c]), np.float32),
            }
            for c in range(NCORES)
        ]

    zero_wc = np.zeros((F, 2), np.float32)

    r1 = agg_layer(x, W1)
    t_total += r1.exec_time_ns or 0
    kernel.launch_times_ns.append(r1.exec_time_ns)
    r2 = _run(tr_mid, transform_maps(r1, g1, be1, zero_wc))
    t_total += r2.exec_time_ns or 0
    kernel.launch_times_ns.append(r2.exec_time_ns)
    h1_full = np.concatenate(
        [r2.results[c]["hpost"][: REAL[c]] for c in range(NCORES)], axis=0
    )
    r3 = agg_layer(h1_full, W2)
    t_total += r3.exec_time_ns or 0
    kernel.launch_times_ns.append(r3.exec_time_ns)
    r4 = _run(tr_end, transform_maps(r3, g2, be2, Wc))
    t_total += r4.exec_time_ns or 0
    kernel.launch_times_ns.append(r4.exec_time_ns)

    y = sum(np.asarray(r4.results[c]["y"], np.float64) for c in range(NCORES))
    out = (y / float(N) + np.asarray(bc, np.float64)).astype(np.float32)
    kernel.last_exec_time_ns = t_total
    return out


# revision 8
# speedup vs baseline: 1.1952x; 1.0004x over previous
"""GraphConv x2 + BN + ReLU + mean-pool + classifier on 8 TRN2 cores.

Strategy (dst-sharded nodes, segment-sum as one-hot matmul):
  - Nodes are split into 8 contiguous blocks of 12500 (padded to 12544 =
    98 chunks x 128).  Each core owns the edges whose dst falls in its block
    (edge-cut partitioning by dst).
  - Edges per core are grouped by 128-node dst-chunk, sorted by src inside
    the chunk, padded per-chunk to T=18 subchunks of 128 edges.
  - Aggregation m^T[feat, seg] += G^T S per 128-edge subchunk:
      G   [128 edges, 64] gathered rows of the (replicated) feature table
      S   [128 edges, 128 segs] one-hot built on DVE from iota==seg, scaled
          by w_e = rsqrt(deg_out[src]) * rsqrt(deg_in[dst])  (norm='both')
    so PSUM accumulates the normalized message sum transposed.
  - Per chunk: h^T = W^T m^T via a second matmul (the conv bias is
    dropped: BatchNorm right after is shift-invariant); BN partial sums;
    h^T written to HBM (pre-BN).
  - BatchNorm needs global stats -> separate transform launch per layer:
    reduces the 8 cores' partials, applies relu(a*h + c), transposes to
    row-major for the next layer's gather (or mean-pool + classifier at
    the end).
  - Host work between launches is routing only (concat / slicing);
    final output = sum of per-core partial logits / N + bc.

Launches: L1 agg(x, W1) -> L2 transform1 -> L3 agg(h1, W2) -> L4
transform2+readout.  Conv biases b1/b2 cancel inside BatchNorm; bc is added
on the host along with the cross-core logit sum (pure routing + 2 adds).
"""
import sys

import numpy as np

sys.path.insert(0, "/opt/trn_rl_repo")

import concourse.bacc as bacc
import concourse.mybir as mybir
import concourse.tile as tile
from concourse.bass import IndirectOffsetOnAxis
from concourse.masks import make_identity

dt = mybir.dt

# ---- problem constants (fixed by the harness) ----
N = 100_000
E = 1_600_000
F = 64
NCORES = 8
P = 128
NPC = 12_500          # nodes per core
CH = 98               # 128-node chunks per core (98*128 = 12544)
NPAD = CH * P         # padded nodes per core
T = 18                # subchunks (of 128 edges) per chunk
EPS = 1e-5
SEG_PAD = 10_000.0    # seg id for pad edges (never matches iota 0..127)

_trace = {"on": False}


def _run(nc, in_maps, trace=None):
    from concourse.bass_utils import run_bass_kernel_spmd

    use_trace = _trace["on"] if trace is None else trace
    if use_trace:
        try:
            import ntff_hook

            ntff_hook.install()
        except Exception:
            use_trace = False
    res = run_bass_kernel_spmd(
        nc,
        in_maps,
        list(range(NCORES)),
        trace=use_trace,
        trace_cores=[0] if use_trace else None,
    )
    return res


# --------------------------------------------------------------------------
# Launch builders
# --------------------------------------------------------------------------

def build_agg(nc_cache={}):
    """Aggregation launch: gather + segment-matmul + W matmul + stat partials.

    Inputs per core:
      xin  [N_ROWS, 64] f32   feature table (replicated, padded rows)
      idx  [128, CH*T] i32    src id of edge (subchunk t, lane p)
      seg  [128, CH*T] f32    dst-local seg id (0..127) or SEG_PAD
      w    [128, CH*T] f32    edge weight (0 for pad)
      Wt   [64, 64]  f32      layer weight
    Outputs:
      hpreT [64, NPAD] f32    pre-BN h, transposed (channels on partitions)
      stats [64, 2]   f32     [sum, sumsq] over this core's nodes
                              (pad columns are exact zeros)
    """
    if "agg" in nc_cache:
        return nc_cache["agg"]
    NROWS = N + 352  # 100352, multiple of 128
    nc = bacc.Bacc("TRN2", target_bir_lowering=False, debug=False)
    xin = nc.dram_tensor("xin", [NROWS, F], dt.float32, kind="ExternalInput")
    idx = nc.dram_tensor("idx", [P, CH * T], dt.int32, kind="ExternalInput")
    seg = nc.dram_tensor("seg", [P, CH * T], dt.float32, kind="ExternalInput")
    w = nc.dram_tensor("w", [P, CH * T], dt.float32, kind="ExternalInput")
    Wt = nc.dram_tensor("Wt", [F, F], dt.float32, kind="ExternalInput")
    hpreT = nc.dram_tensor("hpreT", [F, NPAD], dt.float32, kind="ExternalOutput")
    stats = nc.dram_tensor("stats", [F, 2], dt.float32, kind="ExternalOutput")

    with tile.TileContext(nc) as tc:
        with (
            tc.tile_pool(name="cp", bufs=1) as cp,
            tc.tile_pool(name="gp", bufs=6) as gp,
            tc.tile_pool(name="sp", bufs=4) as sp,
            tc.tile_pool(name="ep", bufs=2) as ep,
            tc.tile_pool(name="pp", bufs=3, space="PSUM") as pp,
        ):
            idx_t = cp.tile([P, CH * T], dt.int32)
            nc.sync.dma_start(out=idx_t[:], in_=idx[:])
            seg_t = cp.tile([P, CH * T], dt.float32)
            nc.sync.dma_start(out=seg_t[:], in_=seg[:])
            w_t = cp.tile([P, CH * T], dt.float32)
            nc.sync.dma_start(out=w_t[:], in_=w[:])
            W_t = cp.tile([F, F], dt.float32)
            nc.sync.dma_start(out=W_t[:], in_=Wt[:])

            iota_i = cp.tile([P, P], dt.int32)
            nc.gpsimd.iota(iota_i[:], pattern=[[1, P]], base=0, channel_multiplier=0)
            iota_f = cp.tile([P, P], dt.float32)
            nc.vector.tensor_copy(out=iota_f[:], in_=iota_i[:])

            sum_sb = cp.tile([F, CH], dt.float32)
            sq_sb = cp.tile([F, CH], dt.float32)

            for g in range(CH):
                G = gp.tile([P, T, F], dt.float32, tag="G")
                for t in range(T):
                    nc.gpsimd.indirect_dma_start(
                        out=G[:, t, :],
                        out_offset=None,
                        in_=xin[:],
                        in_offset=IndirectOffsetOnAxis(
                            ap=idx_t[:, g * T + t : g * T + t + 1], axis=0
                        ),
                    )
                mT_ps = pp.tile([F, P], dt.float32, tag="mT")
                for t in range(T):
                    S = sp.tile([P, P], dt.float32, tag="S")
                    nc.vector.tensor_scalar(
                        out=S[:],
                        in0=iota_f[:],
                        scalar1=seg_t[:, g * T + t : g * T + t + 1],
                        scalar2=w_t[:, g * T + t : g * T + t + 1],
                        op0=mybir.AluOpType.is_equal,
                        op1=mybir.AluOpType.mult,
                    )
                    nc.tensor.matmul(
                        out=mT_ps[:],
                        lhsT=G[:, t, :],
                        rhs=S[:],
                        start=(t == 0),
                        stop=(t == T - 1),
                    )
                mT_sb = ep.tile([F, P], dt.float32, tag="mTsb")
                nc.vector.tensor_copy(out=mT_sb[:], in_=mT_ps[:])
                hT_ps = pp.tile([F, P], dt.float32, tag="hT")
                nc.tensor.matmul(
                    out=hT_ps[:], lhsT=W_t[:], rhs=mT_sb[:], start=True, stop=True
                )
                # h = W^T m  (conv bias is BN-shift-invariant: dropped).
                # Pad node columns are exactly zero, so stats need no mask.
                hT_sb = ep.tile([F, P], dt.float32, tag="hTsb")
                nc.vector.tensor_copy(out=hT_sb[:], in_=hT_ps[:])
                nc.vector.reduce_sum(
                    out=sum_sb[:, g : g + 1], in_=hT_sb[:],
                    axis=mybir.AxisListType.X,
                )
                sq_scr = ep.tile([F, P], dt.float32, tag="sq")
                nc.scalar.activation(
                    out=sq_scr[:],
                    in_=hT_sb[:],
                    func=mybir.ActivationFunctionType.Square,
                    accum_out=sq_sb[:, g : g + 1],
                )
                nc.sync.dma_start(
                    out=hpreT[:, g * P : g * P + P], in_=hT_sb[:]
                )

            stat_sb = cp.tile([F, 2], dt.float32)
            nc.vector.reduce_sum(
                out=stat_sb[:, 0:1], in_=sum_sb[:], axis=mybir.AxisListType.X
            )
            nc.vector.reduce_sum(
                out=stat_sb[:, 1:2], in_=sq_sb[:], axis=mybir.AxisListType.X
            )
            nc.sync.dma_start(out=stats[:], in_=stat_sb[:])

    nc.compile()
    nc_cache["agg"] = nc
    return nc


def build_transform(readout, nc_cache={}):
    """Transform launch: global BN stats -> relu(a*h+c).

    readout=False: output hpost [NPAD, 64] row-major (for next gather).
    readout=True:  output y [1, 2] partial logits (sum_own relu(...) @ Wc).

    Inputs per core:
      hT   [64, NPAD] f32   own pre-BN h (transposed)
      sall [64, 16]  f32    8 cores' [sum, sumsq] partials, interleaved
      gb   [64, 2]   f32    gamma, beta
      Wc   [64, 2]   f32    classifier weight (readout only; else ignored)
    """
    key = ("tr", readout)
    if key in nc_cache:
        return nc_cache[key]
    nc = bacc.Bacc("TRN2", target_bir_lowering=False, debug=False)
    hT = nc.dram_tensor("hT", [F, NPAD], dt.float32, kind="ExternalInput")
    sall = nc.dram_tensor("sall", [F, 2 * NCORES], dt.float32, kind="ExternalInput")
    gb = nc.dram_tensor("gb", [F, 2], dt.float32, kind="ExternalInput")
    Wc = nc.dram_tensor("Wc", [F, 2], dt.float32, kind="ExternalInput")
    padc = nc.dram_tensor("padc", [F, 1], dt.float32, kind="ExternalInput")
    if readout:
        yout = nc.dram_tensor("y", [1, 2], dt.float32, kind="ExternalOutput")
    else:
        hpost = nc.dram_tensor("hpost", [NPAD, F], dt.float32, kind="ExternalOutput")

    with tile.TileContext(nc) as tc:
        with (
            tc.tile_pool(name="cp", bufs=1) as cp,
            tc.tile_pool(name="ep", bufs=2) as ep,
            tc.tile_pool(name="pp", bufs=2, space="PSUM") as pp,
        ):
            hT_t = cp.tile([F, NPAD], dt.float32)
            nc.sync.dma_start(out=hT_t[:], in_=hT[:])
            sall_t = cp.tile([F, 2 * NCORES], dt.float32)
            nc.sync.dma_start(out=sall_t[:], in_=sall[:])
            gb_t = cp.tile([F, 2], dt.float32)
            nc.sync.dma_start(out=gb_t[:], in_=gb[:])
            Wc_t = cp.tile([F, 2], dt.float32)
            nc.sync.dma_start(out=Wc_t[:], in_=Wc[:])
            padc_t = cp.tile([F, 1], dt.float32)
            nc.sync.dma_start(out=padc_t[:], in_=padc[:])

            # stats: columns 0..7 sums, 8..15 sumsqs (host packs that way)
            scr = cp.tile([F, 8], dt.float32)
            nc.vector.reduce_sum(
                out=scr[:, 0:1], in_=sall_t[:, :NCORES], axis=mybir.AxisListType.X
            )
            nc.vector.reduce_sum(
                out=scr[:, 1:2], in_=sall_t[:, NCORES:], axis=mybir.AxisListType.X
            )
            inv_n = 1.0 / float(N)
            # mu = sum/N ; msq = sumsq/N ; var = msq - mu^2
            nc.vector.tensor_scalar(
                out=scr[:, 2:3], in0=scr[:, 0:1], scalar1=inv_n, scalar2=None,
                op0=mybir.AluOpType.mult,
            )  # mu
            nc.vector.tensor_scalar(
                out=scr[:, 3:4], in0=scr[:, 1:2], scalar1=inv_n, scalar2=None,
                op0=mybir.AluOpType.mult,
            )  # msq
            musq = cp.tile([F, 1], dt.float32)
            nc.vector.tensor_tensor(
                out=musq[:], in0=scr[:, 2:3], in1=scr[:, 2:3],
                op=mybir.AluOpType.mult,
            )
            var_eps = cp.tile([F, 1], dt.float32)
            nc.vector.tensor_tensor(
                out=var_eps[:], in0=scr[:, 3:4], in1=musq[:],
                op=mybir.AluOpType.subtract,
            )
            nc.vector.tensor_scalar(
                out=var_eps[:], in0=var_eps[:], scalar1=float(EPS), scalar2=None,
                op0=mybir.AluOpType.add,
            )
            std = cp.tile([F, 1], dt.float32)
            nc.scalar.activation(
                out=std[:], in_=var_eps[:], func=mybir.ActivationFunctionType.Sqrt
            )
            inv_std = cp.tile([F, 1], dt.float32)
            nc.vector.reciprocal(out=inv_std[:], in_=std[:])
            a_col = cp.tile([F, 1], dt.float32)
            nc.vector.tensor_tensor(
                out=a_col[:], in0=gb_t[:, 0:1], in1=inv_std[:],
                op=mybir.AluOpType.mult,
            )
            # c = beta - mu*a
            mua = cp.tile([F, 1], dt.float32)
            nc.vector.tensor_tensor(
                out=mua[:], in0=scr[:, 2:3], in1=a_col[:], op=mybir.AluOpType.mult
            )
            c_col = cp.tile([F, 1], dt.float32)
            nc.vector.tensor_tensor(
                out=c_col[:], in0=gb_t[:, 1:2], in1=mua[:],
                op=mybir.AluOpType.subtract,
            )

            hpostT = cp.tile([F, NPAD], dt.float32)
            nc.scalar.activation(
                out=hpostT[:],
                in_=hT_t[:],
                func=mybir.ActivationFunctionType.Relu,
                scale=a_col[:],
                bias=c_col[:],
            )

            if readout:
                # sum over all cols, then subtract pad_count * relu(c)
                acc = cp.tile([F, 1], dt.float32)
                nc.vector.reduce_sum(
                    out=acc[:], in_=hpostT[:], axis=mybir.AxisListType.X
                )
                relu_c = cp.tile([F, 1], dt.float32)
                nc.scalar.activation(
                    out=relu_c[:], in_=c_col[:],
                    func=mybir.ActivationFunctionType.Relu,
                )
                padsum = cp.tile([F, 1], dt.float32)
                nc.vector.tensor_tensor(
                    out=padsum[:], in0=relu_c[:], in1=padc_t[:],
                    op=mybir.AluOpType.mult,
                )
                nc.vector.tensor_tensor(
                    out=acc[:], in0=acc[:], in1=padsum[:],
                    op=mybir.AluOpType.subtract,
                )
                y_ps = pp.tile([1, 2], dt.float32, tag="y")
                nc.tensor.matmul(
                    out=y_ps[:], lhsT=acc[:], rhs=Wc_t[:], start=True, stop=True
                )
                y_sb = cp.tile([1, 2], dt.float32)
                nc.vector.tensor_copy(out=y_sb[:], in_=y_ps[:])
                nc.sync.dma_start(out=yout[:], in_=y_sb[:])
            else:
                ident = cp.tile([F, F], dt.float32)
                make_identity(nc, ident[:])
                GRP = 7  # chunks per output DMA batch (98 = 14*7)
                for b in range(CH // GRP):
                    tr_sb = ep.tile([P, GRP, F], dt.float32, tag="trsb")
                    for j in range(GRP):
                        g = b * GRP + j
                        tr_ps = pp.tile([P, F], dt.float32, tag="tr")
                        nc.tensor.transpose(
                            out=tr_ps[:],
                            in_=hpostT[:, g * P : g * P + P],
                            identity=ident[:],
                        )
                        nc.vector.tensor_copy(out=tr_sb[:, j, :], in_=tr_ps[:])
                    nc.sync.dma_start(
                        out=hpost[b * GRP * P : (b + 1) * GRP * P, :].rearrange(
                            "(j p) f -> p j f", j=GRP
                        ),
                        in_=tr_sb[:],
                    )

    nc.compile()
    nc_cache[key] = nc
    return nc


# --------------------------------------------------------------------------
# Host-side orchestration
# --------------------------------------------------------------------------

def _prep_edges(src, dst):
    """Per-core edge arrays: idx/seg/w tiles [128, CH*T]."""
    deg_out = np.bincount(src, minlength=N).astype(np.float64)
    deg_in = np.bincount(dst, minlength=N).astype(np.float64)
    r_out = 1.0 / np.sqrt(np.maximum(deg_out, 1.0))
    r_in = 1.0 / np.sqrt(np.maximum(deg_in, 1.0))
    w_edge = (r_out[src] * r_in[dst]).astype(np.float32)

    chunk_of = dst // P  # global chunk id (0..781)
    order = np.lexsort((src, chunk_of))
    src_s = src[order]
    dst_s = dst[order]
    w_s = w_edge[order]
    chunk_s = chunk_of[order]

    counts = np.bincount(chunk_s, minlength=NCORES * CH)
    assert counts.max() <= T * P, f"chunk overflow: {counts.max()} > {T * P}"
    bounds = np.concatenate([[0], np.cumsum(counts)])

    per_core = []
    for c in range(NCORES):
        idx_a = np.zeros((CH * T * P,), np.int32)
        seg_a = np.full((CH * T * P,), SEG_PAD, np.float32)
        w_a = np.zeros((CH * T * P,), np.float32)
        for g in range(CH):
            gc = c * CH + g
            lo, hi = bounds[gc], bounds[gc + 1]
            n = hi - lo
            base = g * T * P
            idx_a[base : base + n] = src_s[lo:hi]
            seg_a[base : base + n] = (dst_s[lo:hi] - gc * P).astype(np.float32)
            w_a[base : base + n] = w_s[lo:hi]
        # lay out edge (t, p) -> tile[p, t]
        idx_tile = idx_a.reshape(CH * T, P).T.copy()
        seg_tile = seg_a.reshape(CH * T, P).T.copy()
        w_tile = w_a.reshape(CH * T, P).T.copy()
        per_core.append((idx_tile, seg_tile, w_tile))
    return per_core


def _pad_rows(x):
    NROWS = N + 352
    out = np.zeros((NROWS, F), np.float32)
    out[:N] = x
    return out


REAL = [min(NPAD, N - c * NPAD) for c in range(NCORES)]  # 12544 x7, 12192


def kernel(x, src, dst, W1, b1, g1, be1, W2, b2, g2, be2, Wc, bc):
    x = np.asarray(x, np.float32)
    src = np.asarray(src, np.int32)
    dst = np.asarray(dst, np.int32)
    per_core = _prep_edges(src, dst)

    agg = build_agg()
    tr_mid = build_transform(readout=False)
    tr_end = build_transform(readout=True)
    t_total = 0
    kernel.launch_times_ns = []

    def agg_layer(x_full, Wl):
        xin = _pad_rows(x_full)
        in_maps = []
        for c in range(NCORES):
            idx_t, seg_t, w_t = per_core[c]
            in_maps.append(
                {
                    "xin": xin,
                    "idx": idx_t,
                    "seg": seg_t,
                    "w": w_t,
                    "Wt": np.asarray(Wl, np.float32),
                }
            )
        return _run(agg, in_maps)

    def transform_maps(res_agg, gl, bel, Wc_):
        st = [r["stats"] for r in res_agg.results]
        sall = np.concatenate(
            [np.stack([s[:, 0] for s in st], 1), np.stack([s[:, 1] for s in st], 1)],
            axis=1,
        ).astype(np.float32)
        gbv = np.stack(
            [np.asarray(gl, np.float32), np.asarray(bel, np.float32)], axis=1
        )
        Wcv = np.asarray(Wc_, np.float32)
        return [
            {
                "hT": res_agg.results[c]["hpreT"],
                "sall": sall,
                "gb": gbv,
                "Wc": Wcv,
                "padc": np.full((F, 1), float(NPAD - REAL[c]), np.float32),
            }
            for c in range(NCORES)
        ]

    zero_wc = np.zeros((F, 2), np.float32)

    r1 = agg_layer(x, W1)
    t_total += r1.exec_time_ns or 0
    kernel.launch_times_ns.append(r1.exec_time_ns)
    r2 = _run(tr_mid, transform_maps(r1, g1, be1, zero_wc))
    t_total += r2.exec_time_ns or 0
    kernel.launch_times_ns.append(r2.exec_time_ns)
    h1_full = np.concatenate(
        [r2.results[c]["hpost"][: REAL[c]] for c in range(NCORES)], axis=0
    )
    r3 = agg_layer(h1_full, W2)
    t_total += r3.exec_time_ns or 0
    kernel.launch_times_ns.append(r3.exec_time_ns)
    r4 = _run(tr_end, transform_maps(r3, g2, be2, Wc))
    t_total += r4.exec_time_ns or 0
    kernel.launch_times_ns.append(r4.exec_time_ns)

    y = sum(np.asarray(r4.results[c]["y"], np.float64) for c in range(NCORES))
    out = (y / float(N) + np.asarray(bc, np.float64)).astype(np.float32)
    kernel.last_exec_time_ns = t_total
    return out


# revision 10
# speedup vs baseline: 1.2618x; 1.0558x over previous
"""GraphConv x2 + BN + ReLU + mean-pool + classifier on 8 TRN2 cores.

Strategy (dst-sharded nodes, segment-sum as one-hot matmul):
  - Nodes are split into 8 contiguous blocks of 12500 (padded to 12544 =
    98 chunks x 128).  Each core owns the edges whose dst falls in its block
    (edge-cut partitioning by dst).
  - Edges per core are grouped by 128-node dst-chunk, sorted by src inside
    the chunk, padded per-chunk to T=18 subchunks of 128 edges.
  - Aggregation m^T[feat, seg] += G^T S per 128-edge subchunk:
      G   [128 edges, 64] gathered rows of the (replicated) feature table
      S   [128 edges, 128 segs] one-hot built on DVE from iota==seg, scaled
          by w_e = rsqrt(deg_out[src]) * rsqrt(deg_in[dst])  (norm='both')
    so PSUM accumulates the normalized message sum transposed.
  - Per chunk: h^T = W^T m^T via a second matmul (the conv bias is
    dropped: BatchNorm right after is shift-invariant); BN partial sums;
    h^T written to HBM (pre-BN).
  - BatchNorm needs global stats -> separate transform launch per layer:
    reduces the 8 cores' partials, applies relu(a*h + c), transposes to
    row-major for the next layer's gather (or mean-pool + classifier at
    the end).
  - Host work between launches is routing only (concat / slicing);
    final output = sum of per-core partial logits / N + bc.

Launches: L1 agg(x, W1) -> L2 transform1 -> L3 agg(h1, W2) -> L4
transform2+readout.  Conv biases b1/b2 cancel inside BatchNorm; bc is added
on the host along with the cross-core logit sum (pure routing + 2 adds).
"""
import sys

import numpy as np

sys.path.insert(0, "/opt/trn_rl_repo")

import concourse.bacc as bacc
import concourse.mybir as mybir
import concourse.tile as tile
from concourse.bass import IndirectOffsetOnAxis
from concourse.masks import make_identity

dt = mybir.dt

# ---- problem constants (fixed by the harness) ----
N = 100_000
E = 1_600_000
F = 64
NCORES = 8
P = 128
NPC = 12_500          # nodes per core
CH = 98               # 128-node chunks per core (98*128 = 12544)
NPAD = CH * P         # padded nodes per core
T = 17                # subchunks (of 128 edges) per chunk
EPS = 1e-5
SEG_PAD = 10_000.0    # seg id for pad edges (never matches iota 0..127)

_trace = {"on": False}


def _run(nc, in_maps, trace=None):
    from concourse.bass_utils import run_bass_kernel_spmd

    use_trace = _trace["on"] if trace is None else trace
    if use_trace:
        try:
            import ntff_hook

            ntff_hook.install()
        except Exception:
            use_trace = False
    res = run_bass_kernel_spmd(
        nc,
        in_maps,
        list(range(NCORES)),
        trace=use_trace,
        trace_cores=[0] if use_trace else None,
    )
    return res


# --------------------------------------------------------------------------
# Launch builders
# --------------------------------------------------------------------------

def build_agg(nc_cache={}):
    """Aggregation launch: gather + segment-matmul + W matmul + stat partials.

    Inputs per core:
      xin  [N_ROWS, 64] f32   feature table (replicated, padded rows)
      idx  [128, CH*T] i32    src id of edge (subchunk t, lane p)
      seg  [128, CH*T] f32    dst-local seg id (0..127) or SEG_PAD
      w    [128, CH*T] f32    edge weight (0 for pad)
      Wt   [64, 64]  f32      layer weight
    Outputs:
      hpreT [64, NPAD] f32    pre-BN h, transposed (channels on partitions)
      stats [64, 2]   f32     [sum, sumsq] over this core's nodes
                              (pad columns are exact zeros)
    """
    if "agg" in nc_cache:
        return nc_cache["agg"]
    NROWS = N + 352  # 100352, multiple of 128
    nc = bacc.Bacc("TRN2", target_bir_lowering=False, debug=False)
    xin = nc.dram_tensor("xin", [NROWS, F], dt.float32, kind="ExternalInput")
    idx = nc.dram_tensor("idx", [P, CH * T], dt.int32, kind="ExternalInput")
    seg = nc.dram_tensor("seg", [P, CH * T], dt.float32, kind="ExternalInput")
    w = nc.dram_tensor("w", [P, CH * T], dt.float32, kind="ExternalInput")
    Wt = nc.dram_tensor("Wt", [F, F], dt.float32, kind="ExternalInput")
    hpreT = nc.dram_tensor("hpreT", [F, NPAD], dt.float32, kind="ExternalOutput")
    stats = nc.dram_tensor("stats", [F, 2], dt.float32, kind="ExternalOutput")

    with tile.TileContext(nc) as tc:
        with (
            tc.tile_pool(name="cp", bufs=1) as cp,
            tc.tile_pool(name="gp", bufs=6) as gp,
            tc.tile_pool(name="sp", bufs=4) as sp,
            tc.tile_pool(name="ep", bufs=2) as ep,
            tc.tile_pool(name="pp", bufs=3, space="PSUM") as pp,
        ):
            idx_t = cp.tile([P, CH * T], dt.int32)
            nc.sync.dma_start(out=idx_t[:], in_=idx[:])
            seg_t = cp.tile([P, CH * T], dt.float32)
            nc.sync.dma_start(out=seg_t[:], in_=seg[:])
            w_t = cp.tile([P, CH * T], dt.float32)
            nc.sync.dma_start(out=w_t[:], in_=w[:])
            W_t = cp.tile([F, F], dt.float32)
            nc.sync.dma_start(out=W_t[:], in_=Wt[:])

            iota_i = cp.tile([P, P], dt.int32)
            nc.gpsimd.iota(iota_i[:], pattern=[[1, P]], base=0, channel_multiplier=0)
            iota_f = cp.tile([P, P], dt.float32)
            nc.vector.tensor_copy(out=iota_f[:], in_=iota_i[:])

            sum_sb = cp.tile([F, CH], dt.float32)
            sq_sb = cp.tile([F, CH], dt.float32)

            for g in range(CH):
                G = gp.tile([P, T, F], dt.float32, tag="G")
                for t in range(T):
                    nc.gpsimd.indirect_dma_start(
                        out=G[:, t, :],
                        out_offset=None,
                        in_=xin[:],
                        in_offset=IndirectOffsetOnAxis(
                            ap=idx_t[:, g * T + t : g * T + t + 1], axis=0
                        ),
                    )
                mT_ps = pp.tile([F, P], dt.float32, tag="mT")
                for t in range(T):
                    S = sp.tile([P, P], dt.float32, tag="S")
                    nc.vector.tensor_scalar(
                        out=S[:],
                        in0=iota_f[:],
                        scalar1=seg_t[:, g * T + t : g * T + t + 1],
                        scalar2=w_t[:, g * T + t : g * T + t + 1],
                        op0=mybir.AluOpType.is_equal,
                        op1=mybir.AluOpType.mult,
                    )
                    nc.tensor.matmul(
                        out=mT_ps[:],
                        lhsT=G[:, t, :],
                        rhs=S[:],
                        start=(t == 0),
                        stop=(t == T - 1),
                    )
                mT_sb = ep.tile([F, P], dt.float32, tag="mTsb")
                nc.vector.tensor_copy(out=mT_sb[:], in_=mT_ps[:])
                hT_ps = pp.tile([F, P], dt.float32, tag="hT")
                nc.tensor.matmul(
                    out=hT_ps[:], lhsT=W_t[:], rhs=mT_sb[:], start=True, stop=True
                )
                # h = W^T m  (conv bias is BN-shift-invariant: dropped).
                # Pad node columns are exactly zero, so stats need no mask.
                hT_sb = ep.tile([F, P], dt.float32, tag="hTsb")
                nc.vector.tensor_copy(out=hT_sb[:], in_=hT_ps[:])
                nc.vector.reduce_sum(
                    out=sum_sb[:, g : g + 1], in_=hT_sb[:],
                    axis=mybir.AxisListType.X,
                )
                sq_scr = ep.tile([F, P], dt.float32, tag="sq")
                nc.scalar.activation(
                    out=sq_scr[:],
                    in_=hT_sb[:],
                    func=mybir.ActivationFunctionType.Square,
                    accum_out=sq_sb[:, g : g + 1],
                )
                nc.sync.dma_start(
                    out=hpreT[:, g * P : g * P + P], in_=hT_sb[:]
                )

            stat_sb = cp.tile([F, 2], dt.float32)
            nc.vector.reduce_sum(
                out=stat_sb[:, 0:1], in_=sum_sb[:], axis=mybir.AxisListType.X
            )
            nc.vector.reduce_sum(
                out=stat_sb[:, 1:2], in_=sq_sb[:], axis=mybir.AxisListType.X
            )
            nc.sync.dma_start(out=stats[:], in_=stat_sb[:])

    nc.compile()
    nc_cache["agg"] = nc
    return nc


def build_transform(readout, nc_cache={}):
    """Transform launch: global BN stats -> relu(a*h+c).

    readout=False: output hpost [NPAD, 64] row-major (for next gather).
    readout=True:  output y [1, 2] partial logits (sum_own relu(...) @ Wc).

    Inputs per core:
      hT   [64, NPAD] f32   own pre-BN h (transposed)
      sall [64, 16]  f32    8 cores' [sum, sumsq] partials, interleaved
      gb   [64, 2]   f32    gamma, beta
      Wc   [64, 2]   f32    classifier weight (readout only; else ignored)
    """
    key = ("tr", readout)
    if key in nc_cache:
        return nc_cache[key]
    nc = bacc.Bacc("TRN2", target_bir_lowering=False, debug=False)
    hT = nc.dram_tensor("hT", [F, NPAD], dt.float32, kind="ExternalInput")
    sall = nc.dram_tensor("sall", [F, 2 * NCORES], dt.float32, kind="ExternalInput")
    gb = nc.dram_tensor("gb", [F, 2], dt.float32, kind="ExternalInput")
    Wc = nc.dram_tensor("Wc", [F, 2], dt.float32, kind="ExternalInput")
    padc = nc.dram_tensor("padc", [F, 1], dt.float32, kind="ExternalInput")
    if readout:
        yout = nc.dram_tensor("y", [1, 2], dt.float32, kind="ExternalOutput")
    else:
        hpost = nc.dram_tensor("hpost", [NPAD, F], dt.float32, kind="ExternalOutput")

    with tile.TileContext(nc) as tc:
        with (
            tc.tile_pool(name="cp", bufs=1) as cp,
            tc.tile_pool(name="ep", bufs=2) as ep,
            tc.tile_pool(name="pp", bufs=2, space="PSUM") as pp,
        ):
            hT_t = cp.tile([F, NPAD], dt.float32)
            nc.sync.dma_start(out=hT_t[:], in_=hT[:])
            sall_t = cp.tile([F, 2 * NCORES], dt.float32)
            nc.sync.dma_start(out=sall_t[:], in_=sall[:])
            gb_t = cp.tile([F, 2], dt.float32)
            nc.sync.dma_start(out=gb_t[:], in_=gb[:])
            Wc_t = cp.tile([F, 2], dt.float32)
            nc.sync.dma_start(out=Wc_t[:], in_=Wc[:])
            padc_t = cp.tile([F, 1], dt.float32)
            nc.sync.dma_start(out=padc_t[:], in_=padc[:])

            # stats: columns 0..7 sums, 8..15 sumsqs (host packs that way)
            scr = cp.tile([F, 8], dt.float32)
            nc.vector.reduce_sum(
                out=scr[:, 0:1], in_=sall_t[:, :NCORES], axis=mybir.AxisListType.X
            )
            nc.vector.reduce_sum(
                out=scr[:, 1:2], in_=sall_t[:, NCORES:], axis=mybir.AxisListType.X
            )
            inv_n = 1.0 / float(N)
            # mu = sum/N ; msq = sumsq/N ; var = msq - mu^2
            nc.vector.tensor_scalar(
                out=scr[:, 2:3], in0=scr[:, 0:1], scalar1=inv_n, scalar2=None,
                op0=mybir.AluOpType.mult,
            )  # mu
            nc.vector.tensor_scalar(
                out=scr[:, 3:4], in0=scr[:, 1:2], scalar1=inv_n, scalar2=None,
                op0=mybir.AluOpType.mult,
            )  # msq
            musq = cp.tile([F, 1], dt.float32)
            nc.vector.tensor_tensor(
                out=musq[:], in0=scr[:, 2:3], in1=scr[:, 2:3],
                op=mybir.AluOpType.mult,
            )
            var_eps = cp.tile([F, 1], dt.float32)
            nc.vector.tensor_tensor(
                out=var_eps[:], in0=scr[:, 3:4], in1=musq[:],
                op=mybir.AluOpType.subtract,
            )
            nc.vector.tensor_scalar(
                out=var_eps[:], in0=var_eps[:], scalar1=float(EPS), scalar2=None,
                op0=mybir.AluOpType.add,
            )
            std = cp.tile([F, 1], dt.float32)
            nc.scalar.activation(
                out=std[:], in_=var_eps[:], func=mybir.ActivationFunctionType.Sqrt
            )
            inv_std = cp.tile([F, 1], dt.float32)
            nc.vector.reciprocal(out=inv_std[:], in_=std[:])
            a_col = cp.tile([F, 1], dt.float32)
            nc.vector.tensor_tensor(
                out=a_col[:], in0=gb_t[:, 0:1], in1=inv_std[:],
                op=mybir.AluOpType.mult,
            )
            # c = beta - mu*a
            mua = cp.tile([F, 1], dt.float32)
            nc.vector.tensor_tensor(
                out=mua[:], in0=scr[:, 2:3], in1=a_col[:], op=mybir.AluOpType.mult
            )
            c_col = cp.tile([F, 1], dt.float32)
            nc.vector.tensor_tensor(
                out=c_col[:], in0=gb_t[:, 1:2], in1=mua[:],
                op=mybir.AluOpType.subtract,
            )

            hpostT = cp.tile([F, NPAD], dt.float32)
            nc.scalar.activation(
                out=hpostT[:],
                in_=hT_t[:],
                func=mybir.ActivationFunctionType.Relu,
                scale=a_col[:],
                bias=c_col[:],
            )

            if readout:
                # sum over all cols, then subtract pad_count * relu(c)
                acc = cp.tile([F, 1], dt.float32)
                nc.vector.reduce_sum(
                    out=acc[:], in_=hpostT[:], axis=mybir.AxisListType.X
                )
                relu_c = cp.tile([F, 1], dt.float32)
                nc.scalar.activation(
                    out=relu_c[:], in_=c_col[:],
                    func=mybir.ActivationFunctionType.Relu,
                )
                padsum = cp.tile([F, 1], dt.float32)
                nc.vector.tensor_tensor(
                    out=padsum[:], in0=relu_c[:], in1=padc_t[:],
                    op=mybir.AluOpType.mult,
                )
                nc.vector.tensor_tensor(
                    out=acc[:], in0=acc[:], in1=padsum[:],
                    op=mybir.AluOpType.subtract,
                )
                y_ps = pp.tile([1, 2], dt.float32, tag="y")
                nc.tensor.matmul(
                    out=y_ps[:], lhsT=acc[:], rhs=Wc_t[:], start=True, stop=True
                )
                y_sb = cp.tile([1, 2], dt.float32)
                nc.vector.tensor_copy(out=y_sb[:], in_=y_ps[:])
                nc.sync.dma_start(out=yout[:], in_=y_sb[:])
            else:
                ident = cp.tile([F, F], dt.float32)
                make_identity(nc, ident[:])
                GRP = 7  # chunks per output DMA batch (98 = 14*7)
                for b in range(CH // GRP):
                    tr_sb = ep.tile([P, GRP, F], dt.float32, tag="trsb")
                    for j in range(GRP):
                        g = b * GRP + j
                        tr_ps = pp.tile([P, F], dt.float32, tag="tr")
                        nc.tensor.transpose(
                            out=tr_ps[:],
                            in_=hpostT[:, g * P : g * P + P],
                            identity=ident[:],
                        )
                        nc.vector.tensor_copy(out=tr_sb[:, j, :], in_=tr_ps[:])
                    nc.sync.dma_start(
                        out=hpost[b * GRP * P : (b + 1) * GRP * P, :].rearrange(
                            "(j p) f -> p j f", j=GRP
                        ),
                        in_=tr_sb[:],
                    )

    nc.compile()
    nc_cache[key] = nc
    return nc


# --------------------------------------------------------------------------
# Host-side orchestration
# --------------------------------------------------------------------------

def _prep_edges(src, dst):
    """Per-core edge arrays: idx/seg/w tiles [128, CH*T].

    Nodes are permuted within each core (greedy bin-packing by in-degree)
    so every 128-node chunk has <= T*128 edges; the layer-2 gather indices
    are remapped through the permutation (glob_row), so the permutation is
    invisible outside this function.
    """
    deg_out = np.bincount(src, minlength=N).astype(np.float64)
    deg_in = np.bincount(dst, minlength=N).astype(np.float64)
    r_out = 1.0 / np.sqrt(np.maximum(deg_out, 1.0))
    r_in = 1.0 / np.sqrt(np.maximum(deg_in, 1.0))
    w_edge = (r_out[src] * r_in[dst]).astype(np.float32)

    # ---- per-core bin-packing: node -> (chunk, pos) ----
    deg_in_i = np.bincount(dst, minlength=N)
    slot = np.zeros(N, np.int64)  # slot within the owning core (0..NPAD-1)
    for c in range(NCORES):
        lo, hi = c * NPAD, min((c + 1) * NPAD, N)
        nodes = np.arange(lo, hi)
        order = np.argsort(-deg_in_i[nodes], kind="stable")
        bins_sum = np.zeros(CH, np.int64)
        bins_cnt = np.zeros(CH, np.int64)
        chunk_a = np.empty(len(nodes), np.int64)
        pos_a = np.empty(len(nodes), np.int64)
        for v in order:
            open_b = np.where(bins_cnt < P)[0]
            b = open_b[np.argmin(bins_sum[open_b])]
            chunk_a[v] = b
            pos_a[v] = bins_cnt[b]
            bins_cnt[b] += 1
            bins_sum[b] += deg_in_i[nodes[v]]
        assert bins_sum.max() <= T * P, f"bin overflow {bins_sum.max()}"
        slot[nodes] = chunk_a * P + pos_a

    glob_row = (np.arange(N) // NPAD) * NPAD + slot  # node -> h1_full row
    chunk_of = (np.arange(N) // NPAD) * CH + slot // P
    chunk_of = chunk_of[dst]  # global chunk id per edge
    seg_of = (slot % P)[dst].astype(np.float32)
    order = np.lexsort((src, chunk_of))
    src_s = src[order]
    dst_s = dst[order]
    w_s = w_edge[order]
    chunk_s = chunk_of[order]

    seg_s = seg_of[order]
    glob_s = glob_row[src[order]].astype(np.int32)

    counts = np.bincount(chunk_s, minlength=NCORES * CH)
    assert counts.max() <= T * P, f"chunk overflow: {counts.max()} > {T * P}"
    bounds = np.concatenate([[0], np.cumsum(counts)])

    per_core = []
    for c in range(NCORES):
        idx_a = np.zeros((CH * T * P,), np.int32)   # layer-1 gather (x rows)
        idx3_a = np.zeros((CH * T * P,), np.int32)  # layer-2 gather (h1 rows)
        seg_a = np.full((CH * T * P,), SEG_PAD, np.float32)
        w_a = np.zeros((CH * T * P,), np.float32)
        for g in range(CH):
            gc = c * CH + g
            lo, hi = bounds[gc], bounds[gc + 1]
            n = hi - lo
            base = g * T * P
            idx_a[base : base + n] = src_s[lo:hi]
            idx3_a[base : base + n] = glob_s[lo:hi]
            seg_a[base : base + n] = seg_s[lo:hi]
            w_a[base : base + n] = w_s[lo:hi]
        idx_tile = idx_a.reshape(CH * T, P).T.copy()
        idx3_tile = idx3_a.reshape(CH * T, P).T.copy()
        seg_tile = seg_a.reshape(CH * T, P).T.copy()
        w_tile = w_a.reshape(CH * T, P).T.copy()
        per_core.append((idx_tile, idx3_tile, seg_tile, w_tile))
    return per_core


def _pad_rows(x):
    NROWS = N + 352
    out = np.zeros((NROWS, F), np.float32)
    out[: len(x)] = x
    return out


REAL = [min(NPAD, N - c * NPAD) for c in range(NCORES)]  # 12544 x7, 12192


def kernel(x, src, dst, W1, b1, g1, be1, W2, b2, g2, be2, Wc, bc):
    x = np.asarray(x, np.float32)
    src = np.asarray(src, np.int32)
    dst = np.asarray(dst, np.int32)
    per_core = _prep_edges(src, dst)

    agg = build_agg()
    tr_mid = build_transform(readout=False)
    tr_end = build_transform(readout=True)
    t_total = 0
    kernel.launch_times_ns = []

    def agg_layer(x_full, Wl, layer):
        xin = _pad_rows(x_full)
        in_maps = []
        for c in range(NCORES):
            idx1_t, idx3_t, seg_t, w_t = per_core[c]
            in_maps.append(
                {
                    "xin": xin,
                    "idx": idx1_t if layer == 1 else idx3_t,
                    "seg": seg_t,
                    "w": w_t,
                    "Wt": np.asarray(Wl, np.float32),
                }
            )
        return _run(agg, in_maps)

    def transform_maps(res_agg, gl, bel, Wc_):
        st = [r["stats"] for r in res_agg.results]
        sall = np.concatenate(
            [np.stack([s[:, 0] for s in st], 1), np.stack([s[:, 1] for s in st], 1)],
            axis=1,
        ).astype(np.float32)
        gbv = np.stack(
            [np.asarray(gl, np.float32), np.asarray(bel, np.float32)], axis=1
        )
        Wcv = np.asarray(Wc_, np.float32)
        return [
            {
                "hT": res_agg.results[c]["hpreT"],
                "sall": sall,
                "gb": gbv,
                "Wc": Wcv,
                "padc": np.full((F, 1), float(NPAD - REAL[c]), np.float32),
            }
            for c in range(NCORES)
        ]

    zero_wc = np.zeros((F, 2), np.float32)

    r1 = agg_layer(x, W1, layer=1)
    t_total += r1.exec_time_ns or 0
    kernel.launch_times_ns.append(r1.exec_time_ns)
    r2 = _run(tr_mid, transform_maps(r1, g1, be1, zero_wc))
    t_total += r2.exec_time_ns or 0
    kernel.launch_times_ns.append(r2.exec_time_ns)
    # keep ALL NPAD rows per core (node order is core-permuted; the layer-2
    # gather indices already point at permuted rows, pads are never gathered)
    h1_full = np.concatenate(
        [r2.results[c]["hpost"] for c in range(NCORES)], axis=0
    )
    r3 = agg_layer(h1_full, W2, layer=2)
    t_total += r3.exec_time_ns or 0
    kernel.launch_times_ns.append(r3.exec_time_ns)
    r4 = _run(tr_end, transform_maps(r3, g2, be2, Wc))
    t_total += r4.exec_time_ns or 0
    kernel.launch_times_ns.append(r4.exec_time_ns)

    y = sum(np.asarray(r4.results[c]["y"], np.float64) for c in range(NCORES))
    out = (y / float(N) + np.asarray(bc, np.float64)).astype(np.float32)
    kernel.last_exec_time_ns = t_total
    return out


# revision 11
# speedup vs baseline: 1.3481x; 1.0684x over previous
"""GraphConv x2 + BN + ReLU + mean-pool + classifier on 8 TRN2 cores.

Strategy (dst-sharded nodes, segment-sum as one-hot matmul):
  - Nodes are split into 8 contiguous blocks of 12500 (padded to 12544 =
    98 chunks x 128).  Each core owns the edges whose dst falls in its block
    (edge-cut partitioning by dst).
  - Edges per core are grouped by 128-node dst-chunk, sorted by src inside
    the chunk, padded per-chunk to T=18 subchunks of 128 edges.
  - Aggregation m^T[feat, seg] += G^T S per 128-edge subchunk:
      G   [128 edges, 64] gathered rows of the (replicated) feature table
      S   [128 edges, 128 segs] one-hot built on DVE from iota==seg, scaled
          by w_e = rsqrt(deg_out[src]) * rsqrt(deg_in[dst])  (norm='both')
    so PSUM accumulates the normalized message sum transposed.
  - Per chunk: h^T = W^T m^T via a second matmul (the conv bias is
    dropped: BatchNorm right after is shift-invariant); BN partial sums;
    h^T written to HBM (pre-BN).
  - BatchNorm needs global stats -> separate transform launch per layer:
    reduces the 8 cores' partials, applies relu(a*h + c), transposes to
    row-major for the next layer's gather (or mean-pool + classifier at
    the end).
  - Host work between launches is routing only (concat / slicing);
    final output = sum of per-core partial logits / N + bc.

Launches: L1 agg(x, W1) -> L2 transform1 -> L3 agg(h1, W2) -> L4
transform2+readout.  Conv biases b1/b2 cancel inside BatchNorm; bc is added
on the host along with the cross-core logit sum (pure routing + 2 adds).
"""
import sys

import numpy as np

sys.path.insert(0, "/opt/trn_rl_repo")

import concourse.bacc as bacc
import concourse.mybir as mybir
import concourse.tile as tile
from concourse.bass import IndirectOffsetOnAxis
from concourse.masks import make_identity

dt = mybir.dt

# ---- problem constants (fixed by the harness) ----
N = 100_000
E = 1_600_000
F = 64
NCORES = 8
P = 128
NPC = 12_500          # nodes per core
CH = 98               # 128-node chunks per core (98*128 = 12544)
NPAD = CH * P         # padded nodes per core
T = 16                # subchunks (of 128 edges) per chunk
EPS = 1e-5
SEG_PAD = 10_000.0    # seg id for pad edges (never matches iota 0..127)

_trace = {"on": False}


def _run(nc, in_maps, trace=None):
    from concourse.bass_utils import run_bass_kernel_spmd

    use_trace = _trace["on"] if trace is None else trace
    if use_trace:
        try:
            import ntff_hook

            ntff_hook.install()
        except Exception:
            use_trace = False
    res = run_bass_kernel_spmd(
        nc,
        in_maps,
        list(range(NCORES)),
        trace=use_trace,
        trace_cores=[0] if use_trace else None,
    )
    return res


# --------------------------------------------------------------------------
# Launch builders
# --------------------------------------------------------------------------

def build_agg(nc_cache={}):
    """Aggregation launch: gather + segment-matmul + W matmul + stat partials.

    Inputs per core:
      xin  [N_ROWS, 64] f32   feature table (replicated, padded rows)
      idx  [128, CH*T] i32    src id of edge (subchunk t, lane p)
      seg  [128, CH*T] f32    dst-local seg id (0..127) or SEG_PAD
      w    [128, CH*T] f32    edge weight (0 for pad)
      Wt   [64, 64]  f32      layer weight
    Outputs:
      hpreT [64, NPAD] f32    pre-BN h, transposed (channels on partitions)
      stats [64, 2]   f32     [sum, sumsq] over this core's nodes
                              (pad columns are exact zeros)
    """
    if "agg" in nc_cache:
        return nc_cache["agg"]
    NROWS = N + 352  # 100352, multiple of 128
    nc = bacc.Bacc("TRN2", target_bir_lowering=False, debug=False)
    xin = nc.dram_tensor("xin", [NROWS, F], dt.float32, kind="ExternalInput")
    idx = nc.dram_tensor("idx", [P, CH * T], dt.int32, kind="ExternalInput")
    seg = nc.dram_tensor("seg", [P, CH * T], dt.float32, kind="ExternalInput")
    w = nc.dram_tensor("w", [P, CH * T], dt.float32, kind="ExternalInput")
    Wt = nc.dram_tensor("Wt", [F, F], dt.float32, kind="ExternalInput")
    hpreT = nc.dram_tensor("hpreT", [F, NPAD], dt.float32, kind="ExternalOutput")
    stats = nc.dram_tensor("stats", [F, 2], dt.float32, kind="ExternalOutput")

    with tile.TileContext(nc) as tc:
        with (
            tc.tile_pool(name="cp", bufs=1) as cp,
            tc.tile_pool(name="gp", bufs=6) as gp,
            tc.tile_pool(name="sp", bufs=4) as sp,
            tc.tile_pool(name="ep", bufs=2) as ep,
            tc.tile_pool(name="pp", bufs=3, space="PSUM") as pp,
        ):
            idx_t = cp.tile([P, CH * T], dt.int32)
            nc.sync.dma_start(out=idx_t[:], in_=idx[:])
            seg_t = cp.tile([P, CH * T], dt.float32)
            nc.sync.dma_start(out=seg_t[:], in_=seg[:])
            w_t = cp.tile([P, CH * T], dt.float32)
            nc.sync.dma_start(out=w_t[:], in_=w[:])
            W_t = cp.tile([F, F], dt.float32)
            nc.sync.dma_start(out=W_t[:], in_=Wt[:])

            iota_i = cp.tile([P, P], dt.int32)
            nc.gpsimd.iota(iota_i[:], pattern=[[1, P]], base=0, channel_multiplier=0)
            iota_f = cp.tile([P, P], dt.float32)
            nc.vector.tensor_copy(out=iota_f[:], in_=iota_i[:])

            sum_sb = cp.tile([F, CH], dt.float32)
            sq_sb = cp.tile([F, CH], dt.float32)

            for g in range(CH):
                G = gp.tile([P, T, F], dt.float32, tag="G")
                for t in range(T):
                    nc.gpsimd.indirect_dma_start(
                        out=G[:, t, :],
                        out_offset=None,
                        in_=xin[:],
                        in_offset=IndirectOffsetOnAxis(
                            ap=idx_t[:, g * T + t : g * T + t + 1], axis=0
                        ),
                    )
                mT_ps = pp.tile([F, P], dt.float32, tag="mT")
                for t in range(T):
                    S = sp.tile([P, P], dt.float32, tag="S")
                    nc.vector.tensor_scalar(
                        out=S[:],
                        in0=iota_f[:],
                        scalar1=seg_t[:, g * T + t : g * T + t + 1],
                        scalar2=w_t[:, g * T + t : g * T + t + 1],
                        op0=mybir.AluOpType.is_equal,
                        op1=mybir.AluOpType.mult,
                    )
                    nc.tensor.matmul(
                        out=mT_ps[:],
                        lhsT=G[:, t, :],
                        rhs=S[:],
                        start=(t == 0),
                        stop=(t == T - 1),
                    )
                mT_sb = ep.tile([F, P], dt.float32, tag="mTsb")
                nc.vector.tensor_copy(out=mT_sb[:], in_=mT_ps[:])
                hT_ps = pp.tile([F, P], dt.float32, tag="hT")
                nc.tensor.matmul(
                    out=hT_ps[:], lhsT=W_t[:], rhs=mT_sb[:], start=True, stop=True
                )
                # h = W^T m  (conv bias is BN-shift-invariant: dropped).
                # Pad node columns are exactly zero, so stats need no mask.
                hT_sb = ep.tile([F, P], dt.float32, tag="hTsb")
                nc.vector.tensor_copy(out=hT_sb[:], in_=hT_ps[:])
                nc.vector.reduce_sum(
                    out=sum_sb[:, g : g + 1], in_=hT_sb[:],
                    axis=mybir.AxisListType.X,
                )
                sq_scr = ep.tile([F, P], dt.float32, tag="sq")
                nc.scalar.activation(
                    out=sq_scr[:],
                    in_=hT_sb[:],
                    func=mybir.ActivationFunctionType.Square,
                    accum_out=sq_sb[:, g : g + 1],
                )
                nc.sync.dma_start(
                    out=hpreT[:, g * P : g * P + P], in_=hT_sb[:]
                )

            stat_sb = cp.tile([F, 2], dt.float32)
            nc.vector.reduce_sum(
                out=stat_sb[:, 0:1], in_=sum_sb[:], axis=mybir.AxisListType.X
            )
            nc.vector.reduce_sum(
                out=stat_sb[:, 1:2], in_=sq_sb[:], axis=mybir.AxisListType.X
            )
            nc.sync.dma_start(out=stats[:], in_=stat_sb[:])

    nc.compile()
    nc_cache["agg"] = nc
    return nc


def build_transform(readout, nc_cache={}):
    """Transform launch: global BN stats -> relu(a*h+c).

    readout=False: output hpost [NPAD, 64] row-major (for next gather).
    readout=True:  output y [1, 2] partial logits (sum_own relu(...) @ Wc).

    Inputs per core:
      hT   [64, NPAD] f32   own pre-BN h (transposed)
      sall [64, 16]  f32    8 cores' [sum, sumsq] partials, interleaved
      gb   [64, 2]   f32    gamma, beta
      Wc   [64, 2]   f32    classifier weight (readout only; else ignored)
    """
    key = ("tr", readout)
    if key in nc_cache:
        return nc_cache[key]
    nc = bacc.Bacc("TRN2", target_bir_lowering=False, debug=False)
    hT = nc.dram_tensor("hT", [F, NPAD], dt.float32, kind="ExternalInput")
    sall = nc.dram_tensor("sall", [F, 2 * NCORES], dt.float32, kind="ExternalInput")
    gb = nc.dram_tensor("gb", [F, 2], dt.float32, kind="ExternalInput")
    Wc = nc.dram_tensor("Wc", [F, 2], dt.float32, kind="ExternalInput")
    padc = nc.dram_tensor("padc", [F, 1], dt.float32, kind="ExternalInput")
    if readout:
        yout = nc.dram_tensor("y", [1, 2], dt.float32, kind="ExternalOutput")
    else:
        hpost = nc.dram_tensor("hpost", [NPAD, F], dt.float32, kind="ExternalOutput")

    with tile.TileContext(nc) as tc:
        with (
            tc.tile_pool(name="cp", bufs=1) as cp,
            tc.tile_pool(name="ep", bufs=2) as ep,
            tc.tile_pool(name="pp", bufs=2, space="PSUM") as pp,
        ):
            hT_t = cp.tile([F, NPAD], dt.float32)
            nc.sync.dma_start(out=hT_t[:], in_=hT[:])
            sall_t = cp.tile([F, 2 * NCORES], dt.float32)
            nc.sync.dma_start(out=sall_t[:], in_=sall[:])
            gb_t = cp.tile([F, 2], dt.float32)
            nc.sync.dma_start(out=gb_t[:], in_=gb[:])
            Wc_t = cp.tile([F, 2], dt.float32)
            nc.sync.dma_start(out=Wc_t[:], in_=Wc[:])
            padc_t = cp.tile([F, 1], dt.float32)
            nc.sync.dma_start(out=padc_t[:], in_=padc[:])

            # stats: columns 0..7 sums, 8..15 sumsqs (host packs that way)
            scr = cp.tile([F, 8], dt.float32)
            nc.vector.reduce_sum(
                out=scr[:, 0:1], in_=sall_t[:, :NCORES], axis=mybir.AxisListType.X
            )
            nc.vector.reduce_sum(
                out=scr[:, 1:2], in_=sall_t[:, NCORES:], axis=mybir.AxisListType.X
            )
            inv_n = 1.0 / float(N)
            # mu = sum/N ; msq = sumsq/N ; var = msq - mu^2
            nc.vector.tensor_scalar(
                out=scr[:, 2:3], in0=scr[:, 0:1], scalar1=inv_n, scalar2=None,
                op0=mybir.AluOpType.mult,
            )  # mu
            nc.vector.tensor_scalar(
                out=scr[:, 3:4], in0=scr[:, 1:2], scalar1=inv_n, scalar2=None,
                op0=mybir.AluOpType.mult,
            )  # msq
            musq = cp.tile([F, 1], dt.float32)
            nc.vector.tensor_tensor(
                out=musq[:], in0=scr[:, 2:3], in1=scr[:, 2:3],
                op=mybir.AluOpType.mult,
            )
            var_eps = cp.tile([F, 1], dt.float32)
            nc.vector.tensor_tensor(
                out=var_eps[:], in0=scr[:, 3:4], in1=musq[:],
                op=mybir.AluOpType.subtract,
            )
            nc.vector.tensor_scalar(
                out=var_eps[:], in0=var_eps[:], scalar1=float(EPS), scalar2=None,
                op0=mybir.AluOpType.add,
            )
            std = cp.tile([F, 1], dt.float32)
            nc.scalar.activation(
                out=std[:], in_=var_eps[:], func=mybir.ActivationFunctionType.Sqrt
            )
            inv_std = cp.tile([F, 1], dt.float32)
            nc.vector.reciprocal(out=inv_std[:], in_=std[:])
            a_col = cp.tile([F, 1], dt.float32)
            nc.vector.tensor_tensor(
                out=a_col[:], in0=gb_t[:, 0:1], in1=inv_std[:],
                op=mybir.AluOpType.mult,
            )
            # c = beta - mu*a
            mua = cp.tile([F, 1], dt.float32)
            nc.vector.tensor_tensor(
                out=mua[:], in0=scr[:, 2:3], in1=a_col[:], op=mybir.AluOpType.mult
            )
            c_col = cp.tile([F, 1], dt.float32)
            nc.vector.tensor_tensor(
                out=c_col[:], in0=gb_t[:, 1:2], in1=mua[:],
                op=mybir.AluOpType.subtract,
            )

            hpostT = cp.tile([F, NPAD], dt.float32)
            nc.scalar.activation(
                out=hpostT[:],
                in_=hT_t[:],
                func=mybir.ActivationFunctionType.Relu,
                scale=a_col[:],
                bias=c_col[:],
            )

            if readout:
                # sum over all cols, then subtract pad_count * relu(c)
                acc = cp.tile([F, 1], dt.float32)
                nc.vector.reduce_sum(
                    out=acc[:], in_=hpostT[:], axis=mybir.AxisListType.X
                )
                relu_c = cp.tile([F, 1], dt.float32)
                nc.scalar.activation(
                    out=relu_c[:], in_=c_col[:],
                    func=mybir.ActivationFunctionType.Relu,
                )
                padsum = cp.tile([F, 1], dt.float32)
                nc.vector.tensor_tensor(
                    out=padsum[:], in0=relu_c[:], in1=padc_t[:],
                    op=mybir.AluOpType.mult,
                )
                nc.vector.tensor_tensor(
                    out=acc[:], in0=acc[:], in1=padsum[:],
                    op=mybir.AluOpType.subtract,
                )
                y_ps = pp.tile([1, 2], dt.float32, tag="y")
                nc.tensor.matmul(
                    out=y_ps[:], lhsT=acc[:], rhs=Wc_t[:], start=True, stop=True
                )
                y_sb = cp.tile([1, 2], dt.float32)
                nc.vector.tensor_copy(out=y_sb[:], in_=y_ps[:])
                nc.sync.dma_start(out=yout[:], in_=y_sb[:])
            else:
                ident = cp.tile([F, F], dt.float32)
                make_identity(nc, ident[:])
                GRP = 7  # chunks per output DMA batch (98 = 14*7)
                for b in range(CH // GRP):
                    tr_sb = ep.tile([P, GRP, F], dt.float32, tag="trsb")
                    for j in range(GRP):
                        g = b * GRP + j
                        tr_ps = pp.tile([P, F], dt.float32, tag="tr")
                        nc.tensor.transpose(
                            out=tr_ps[:],
                            in_=hpostT[:, g * P : g * P + P],
                            identity=ident[:],
                        )
                        nc.vector.tensor_copy(out=tr_sb[:, j, :], in_=tr_ps[:])
                    nc.sync.dma_start(
                        out=hpost[b * GRP * P : (b + 1) * GRP * P, :].rearrange(
                            "(j p) f -> p j f", j=GRP
                        ),
                        in_=tr_sb[:],
                    )

    nc.compile()
    nc_cache[key] = nc
    return nc


# --------------------------------------------------------------------------
# Host-side orchestration
# --------------------------------------------------------------------------

def _prep_edges(src, dst):
    """Per-core edge arrays: idx/seg/w tiles [128, CH*T].

    Nodes are permuted within each core (greedy bin-packing by in-degree)
    so every 128-node chunk has <= T*128 edges; the layer-2 gather indices
    are remapped through the permutation (glob_row), so the permutation is
    invisible outside this function.
    """
    deg_out = np.bincount(src, minlength=N).astype(np.float64)
    deg_in = np.bincount(dst, minlength=N).astype(np.float64)
    r_out = 1.0 / np.sqrt(np.maximum(deg_out, 1.0))
    r_in = 1.0 / np.sqrt(np.maximum(deg_in, 1.0))
    w_edge = (r_out[src] * r_in[dst]).astype(np.float32)

    # ---- cross-core rebalance + per-core bin-packing ----
    deg_in_i = np.bincount(dst, minlength=N)
    core_of = (np.arange(N) // NPAD).astype(np.int64)
    LIMIT = CH * (T * P - 4)  # per-core edge budget with packing slack
    totals = np.bincount(core_of, weights=deg_in_i.astype(np.float64),
                         minlength=NCORES).astype(np.int64)
    ccnt = np.bincount(core_of, minlength=NCORES)
    for c in range(NCORES):
        if totals[c] <= LIMIT:
            continue
        nodes_c = np.where(core_of == c)[0]
        for v in nodes_c[np.argsort(-deg_in_i[nodes_c], kind="stable")]:
            if totals[c] <= LIMIT:
                break
            cand = [t for t in range(NCORES)
                    if ccnt[t] < NPAD and totals[t] + deg_in_i[v] <= LIMIT]
            if not cand:
                break
            tgt = min(cand, key=lambda t: totals[t])
            core_of[v] = tgt
            totals[c] -= deg_in_i[v]
            totals[tgt] += deg_in_i[v]
            ccnt[c] -= 1
            ccnt[tgt] += 1
    assert totals.max() <= CH * T * P, f"core overflow {totals.max()}"

    slot = np.zeros(N, np.int64)  # slot within the owning core (0..NPAD-1)
    for c in range(NCORES):
        nodes = np.where(core_of == c)[0]
        order = np.argsort(-deg_in_i[nodes], kind="stable")
        bins_sum = np.zeros(CH, np.int64)
        bins_cnt = np.zeros(CH, np.int64)
        members = [[] for _ in range(CH)]
        for v in order:
            open_b = np.where(bins_cnt < P)[0]
            b = open_b[np.argmin(bins_sum[open_b])]
            members[b].append(v)
            bins_cnt[b] += 1
            bins_sum[b] += deg_in_i[nodes[v]]
        LIM = T * P
        for _ in range(5000):  # swap refinement
            bhi = int(np.argmax(bins_sum))
            if bins_sum[bhi] <= LIM:
                break
            du = deg_in_i[nodes[members[bhi]]]
            moved = False
            for blo in np.argsort(bins_sum):
                head = LIM - bins_sum[blo]
                if blo == bhi or head <= 0:
                    continue
                dv = deg_in_i[nodes[members[blo]]]
                cand = du[:, None].astype(np.int64) - dv[None, :]
                cand[cand > head] = -1
                ui, vj = np.unravel_index(np.argmax(cand), cand.shape)
                delta = cand[ui, vj]
                if delta >= 1:
                    u = members[bhi][ui]
                    v2 = members[blo][vj]
                    members[bhi][ui] = v2
                    members[blo][vj] = u
                    bins_sum[bhi] -= delta
                    bins_sum[blo] += delta
                    moved = True
                    break
            if not moved:
                break
        assert bins_sum.max() <= LIM, f"bin overflow {bins_sum.max()}"
        for b in range(CH):
            for j, v in enumerate(members[b]):
                slot[nodes[v]] = b * P + j

    pad_counts = [int(NPAD - ccnt[c]) for c in range(NCORES)]
    glob_row = core_of * NPAD + slot  # node -> h1_full row
    chunk_of = core_of * CH + slot // P
    chunk_of = chunk_of[dst]  # global chunk id per edge
    seg_of = (slot % P)[dst].astype(np.float32)
    order = np.lexsort((src, chunk_of))
    src_s = src[order]
    dst_s = dst[order]
    w_s = w_edge[order]
    chunk_s = chunk_of[order]

    seg_s = seg_of[order]
    glob_s = glob_row[src[order]].astype(np.int32)

    counts = np.bincount(chunk_s, minlength=NCORES * CH)
    assert counts.max() <= T * P, f"chunk overflow: {counts.max()} > {T * P}"
    bounds = np.concatenate([[0], np.cumsum(counts)])

    per_core = []
    for c in range(NCORES):
        idx_a = np.zeros((CH * T * P,), np.int32)   # layer-1 gather (x rows)
        idx3_a = np.zeros((CH * T * P,), np.int32)  # layer-2 gather (h1 rows)
        seg_a = np.full((CH * T * P,), SEG_PAD, np.float32)
        w_a = np.zeros((CH * T * P,), np.float32)
        for g in range(CH):
            gc = c * CH + g
            lo, hi = bounds[gc], bounds[gc + 1]
            n = hi - lo
            base = g * T * P
            idx_a[base : base + n] = src_s[lo:hi]
            idx3_a[base : base + n] = glob_s[lo:hi]
            seg_a[base : base + n] = seg_s[lo:hi]
            w_a[base : base + n] = w_s[lo:hi]
        idx_tile = idx_a.reshape(CH * T, P).T.copy()
        idx3_tile = idx3_a.reshape(CH * T, P).T.copy()
        seg_tile = seg_a.reshape(CH * T, P).T.copy()
        w_tile = w_a.reshape(CH * T, P).T.copy()
        per_core.append((idx_tile, idx3_tile, seg_tile, w_tile))
    return per_core, pad_counts


def _pad_rows(x):
    NROWS = N + 352
    out = np.zeros((NROWS, F), np.float32)
    out[: len(x)] = x
    return out


REAL = [min(NPAD, N - c * NPAD) for c in range(NCORES)]  # 12544 x7, 12192


def kernel(x, src, dst, W1, b1, g1, be1, W2, b2, g2, be2, Wc, bc):
    x = np.asarray(x, np.float32)
    src = np.asarray(src, np.int32)
    dst = np.asarray(dst, np.int32)
    per_core, pad_counts = _prep_edges(src, dst)

    agg = build_agg()
    tr_mid = build_transform(readout=False)
    tr_end = build_transform(readout=True)
    t_total = 0
    kernel.launch_times_ns = []

    def agg_layer(x_full, Wl, layer):
        xin = _pad_rows(x_full)
        in_maps = []
        for c in range(NCORES):
            idx1_t, idx3_t, seg_t, w_t = per_core[c]
            in_maps.append(
                {
                    "xin": xin,
                    "idx": idx1_t if layer == 1 else idx3_t,
                    "seg": seg_t,
                    "w": w_t,
                    "Wt": np.asarray(Wl, np.float32),
                }
            )
        return _run(agg, in_maps)

    def transform_maps(res_agg, gl, bel, Wc_):
        st = [r["stats"] for r in res_agg.results]
        sall = np.concatenate(
            [np.stack([s[:, 0] for s in st], 1), np.stack([s[:, 1] for s in st], 1)],
            axis=1,
        ).astype(np.float32)
        gbv = np.stack(
            [np.asarray(gl, np.float32), np.asarray(bel, np.float32)], axis=1
        )
        Wcv = np.asarray(Wc_, np.float32)
        return [
            {
                "hT": res_agg.results[c]["hpreT"],
                "sall": sall,
                "gb": gbv,
                "Wc": Wcv,
                "padc": np.full((F, 1), float(pad_counts[c]), np.float32),
            }
            for c in range(NCORES)
        ]

    zero_wc = np.zeros((F, 2), np.float32)

    r1 = agg_layer(x, W1, layer=1)
    t_total += r1.exec_time_ns or 0
    kernel.launch_times_ns.append(r1.exec_time_ns)
    r2 = _run(tr_mid, transform_maps(r1, g1, be1, zero_wc))
    t_total += r2.exec_time_ns or 0
    kernel.launch_times_ns.append(r2.exec_time_ns)
    # keep ALL NPAD rows per core (node order is core-permuted; the layer-2
    # gather indices already point at permuted rows, pads are never gathered)
    h1_full = np.concatenate(
        [r2.results[c]["hpost"] for c in range(NCORES)], axis=0
    )
    r3 = agg_layer(h1_full, W2, layer=2)
    t_total += r3.exec_time_ns or 0
    kernel.launch_times_ns.append(r3.exec_time_ns)
    r4 = _run(tr_end, transform_maps(r3, g2, be2, Wc))
    t_total += r4.exec_time_ns or 0
    kernel.launch_times_ns.append(r4.exec_time_ns)

    y = sum(np.asarray(r4.results[c]["y"], np.float64) for c in range(NCORES))
    out = (y / float(N) + np.asarray(bc, np.float64)).astype(np.float32)
    kernel.last_exec_time_ns = t_total
    return out
